# revision 13
# baseline (speedup 1.0000x reference)
"""Trainium2 Bass kernel for nn_BinarizedCifar10MLP — v3.

Data-parallel over batch (8192/8 = 1024 rows/core), feature-major layout.

vs v2 (573us):
  - All weight signing moved to the HOST: W1 ships as fp8e4 +-1 (6.3MB,
    was 12.6MB bf16 + on-device sign), W2/W3 ship as fp8e4 +-1 in DR
    layout (no bf16 read + sign + DRAM round-trip prepass at all).
  - DMA queue discipline: bulk loads (x, W1, Wdr) ride the ACT hwdge
    queue; the sync queue carries only AR traffic + consts + final out.
    W1 m0 is FIRST on the queue (v2 had it behind all 9.4MB of x -> 43us
    PE stall at start); x chunks interleave with W1 m0 sub-tiles and the
    L1 m-loop consumes x chunk-by-chunk, so the PE rides the DMA wave.
  - 3-chunk BN-stat AllReduce for L2/L3 (m 0-9 / 10-13 / 14-15): AR-A
    fires at ~60% of the (short) layer instead of 87%, landing before
    the layer ends; quad-grouped phase-major matmul emission gives the
    PE a 4-m-tile runway on already-signed k-tiles while the tail AR
    lands. Stat sums are n-pair-reduced BEFORE the AR (half payload).
  - log-softmax tail in 4 chunks of 256 cols, exp computed straight
    from PSUM in parallel with the logits drain (DVE reads PSUM).
"""

import sys

sys.path.insert(0, "/opt/trn_rl_repo")

import numpy as np
import ml_dtypes

B, D, H, C = 8192, 3 * 32 * 32, 2048, 10
EPS = 1e-5
NCORES = 8
BS = B // NCORES          # 1024 batch rows per core
KD = D // 128             # 24 k-tiles over input dim
KH = H // 128             # 16 k-tiles over hidden dim
NB = BS // 512            # 2 free-dim chunks of 512
CHK = {1: (14, 16), 2: (10, 14, 16), 3: (10, 14, 16)}  # AR chunk end bounds
XCH = 4                   # x DMA chunks (6 k-tiles each)
KC = KD // XCH            # 6 k-tiles per x chunk

_CACHE = {}


def _bounds(l):
    return (0,) + CHK[l]


def _u_ks(l_prod):
    """k-tiles of the layer-l_prod sign output that are {0,1}-encoded (single
    DVE is_ge op); the consumer weights are host-scaled to +-2 on those blocks
    with the -rowsum(sign W) correction folded into the bias."""
    bd = _bounds(l_prod)
    nch = len(bd) - 1
    dve = {nch - 1} if l_prod == 1 else set(range(1, nch))
    ks = set()
    for ci in range(nch):
        for k in range(bd[ci], bd[ci + 1]):
            if ci in dve or k % 2 == 0:
                ks.add(k)
    return ks


def _build(stage=7, fast=(False, False)):
    import concourse.bacc as bacc
    import concourse.mybir as mybir
    import concourse.tile as tile

    F32 = mybir.dt.float32
    F16 = mybir.dt.float16
    F8E4 = mybir.dt.float8e4
    F8E5 = mybir.dt.float8e5
    DRM = mybir.MatmulPerfMode.DoubleRow
    ACT = mybir.ActivationFunctionType
    ALU = mybir.AluOpType
    RG = [list(range(NCORES))]

    nc = bacc.Bacc("TRN2", target_bir_lowering=False, debug=False, num_devices=NCORES)

    # ---- I/O ----
    xhi_d = nc.dram_tensor("xT_hi", [128, KD * BS], F16, kind="ExternalInput").ap()
    xlo_d = nc.dram_tensor("xT_lo8", [128, KD * BS], F8E4, kind="ExternalInput").ap()
    w1_d = nc.dram_tensor("w1pk", [128, KH * KD * 128], F8E4, kind="ExternalInput").ap()
    w2_d = nc.dram_tensor("w2pk", [128, KH * KH * 128], F8E4, kind="ExternalInput").ap()
    w3_d = nc.dram_tensor("w3pk", [128, KH * KH * 128], F8E4, kind="ExternalInput").ap()
    CNAMES = ("b1", "g1", "bt1", "b2", "g2", "bt2", "b3", "g3", "bt3")
    cpk_d = nc.dram_tensor("cpk", [128, KH * len(CNAMES)], F32, kind="ExternalInput").ap()
    w4pk_d = nc.dram_tensor("w4pk", [128, C * KH], F16, kind="ExternalInput").ap()
    b4_d = nc.dram_tensor("c_b4", [16, 1], F32, kind="ExternalInput").ap()
    out_d = nc.dram_tensor("outT", [C, BS], F32, kind="ExternalOutput").ap()

    wl_d = {2: w2_d, 3: w3_d}

    with tile.TileContext(nc) as tc:
        with (
            tc.tile_pool(name="pconst", bufs=1) as pconst,
            tc.tile_pool(name="pstat", bufs=1) as pstat,
            tc.tile_pool(name="plog", bufs=1) as plog,
            tc.tile_pool(name="ptail", bufs=6) as ptail,
            tc.tile_pool(name="pscr", bufs=3) as pscr,
            tc.tile_pool(name="pw", bufs=4) as pw,
            tc.tile_pool(name="pw8", bufs=2) as pw8,
            tc.tile_pool(name="pwdr", bufs=8) as pwdr,
            tc.tile_pool(name="py3", bufs=4) as py3,
            tc.tile_pool(name="ph", bufs=1) as ph,
            tc.tile_pool(name="pa", bufs=1) as pa,
            tc.tile_pool(name="pb", bufs=1) as pb,
            tc.tile_pool(name="ppsum", bufs=8, space="PSUM") as ppsum,
            tc.tile_pool(name="pdram", bufs=16, space="DRAM") as pdram,
        ):
            # ---- warmup AllReduce: absorbs ncfw first-collective staging ----
            wuin = pdram.tile([128, 4], F32, tag="wuin")
            wuout = pdram.tile([128, 4], F32, tag="wuout")
            wusrc = pstat.tile([128, 4], F32, tag="wusrc")
            nc.vector.memset(wusrc[:], 0.0)
            nc.sync.dma_start(wuin[:], wusrc[:])
            nc.gpsimd.collective_compute(
                "AllReduce", ALU.add, replica_groups=RG,
                ins=[wuin.opt()], outs=[wuout.opt()])

            def warm_ar():
                # keep the CC stream hot: cold-picked collectives take ~30us
                # vs ~9us warm
                nc.gpsimd.collective_compute(
                    "AllReduce", ALU.add, replica_groups=RG,
                    ins=[wuin.opt()], outs=[wuout.opt()])

            # ---- constants (sync queue; small, land early) ----
            cpk = pconst.tile([128, KH * len(CNAMES)], F32, tag="cpk")
            nc.sync.dma_start(cpk[:], cpk_d)
            cons = {name: cpk[:, i * KH:(i + 1) * KH] for i, name in enumerate(CNAMES)}
            b4s = pconst.tile([16, 1], F32, tag="b4")
            nc.sync.dma_start(b4s[:], b4_d)
            ones10 = pconst.tile([16, 1], F32, tag="ones10")
            nc.vector.memset(ones10[:], 1.0)
            onesC = pconst.tile([1, 16], F32, tag="onesC")
            nc.vector.memset(onesC[:], 1.0)

            # ---- bulk loads: scalar (ACT hwdge) queue ----
            # W1 m0 interleaved chunk-wise with x so the PE starts ~5us in.
            xhi = pa.tile([128, KD * BS], F16, tag="pa", name="xhi")
            xlo8 = pb.tile([128, KD * BS], F8E4, tag="pb", name="xlo8")
            w1_pf = {}
            for m in range(3):
                w1_pf[m] = pw.tile([128, KD * 128], F8E4, tag="w", name=f"w1_{m}")
            for c in range(XCH):
                nc.scalar.dma_start(
                    w1_pf[0][:, c * KC * 128:(c + 1) * KC * 128],
                    w1_d[:, c * KC * 128:(c + 1) * KC * 128])
                sl = slice(c * KC * BS, (c + 1) * KC * BS)
                nc.scalar.dma_start(xhi[:, sl], xhi_d[:, sl])
                nc.scalar.dma_start(xlo8[:, sl], xlo_d[:, sl])
            for m in range(1, 3):
                nc.scalar.dma_start(
                    w1_pf[m][:], w1_d[:, m * KD * 128:(m + 1) * KD * 128])
            xlo8v = xlo8[:].rearrange("p (k c) -> p k c", c=BS)

            # Wdr stream: fp8 +-1 DR-layout weights for L2/L3, 8-deep ring.
            # gens 0..15 = L2 m0..15, 16..31 = L3 m0..15.
            wdr_pf = {}

            def emit_wdr(gen):
                l, m = (2, gen) if gen < 16 else (3, gen - 16)
                w8t = pwdr.tile([128, KH * 128], F8E4, tag="wdr", name=f"wdr_{l}_{m}")
                nc.scalar.dma_start(w8t[:], wl_d[l][:, m * 2048:(m + 1) * 2048])
                wdr_pf[(l, m)] = w8t

            for gen in range(3):
                emit_wdr(gen)

            w4f = pconst.tile([128, C * KH], F16, tag="w4f")
            nc.scalar.dma_start(w4f[:], w4pk_d)

            parts = {}
            gchunk = {}     # (l, ci) -> allreduced pre-reduced stats tile
            stats = {}      # (l, ci) -> dict of stat tiles
            arouts = {}

            def emit_ar_fire(l, ci, do_sq):
                """n-pair-reduce parts chunk -> DRAM -> AllReduce."""
                bd = _bounds(l)
                c0, c1 = bd[ci], bd[ci + 1]
                nm = c1 - c0
                w = 2 * nm if do_sq else nm
                red = pstat.tile([128, w], F32, tag=f"red{l}{ci}", name=f"red{l}{ci}")
                nc.vector.tensor_reduce(
                    red[:, 0:nm],
                    parts[l][:, 2 * c0:2 * c1].rearrange("p (m n) -> p m n", n=2),
                    axis=mybir.AxisListType.X, op=ALU.add)
                if do_sq:
                    nc.vector.tensor_reduce(
                        red[:, nm:w],
                        parts[l][:, 32 + 2 * c0:32 + 2 * c1]
                        .rearrange("p (m n) -> p m n", n=2),
                        axis=mybir.AxisListType.X, op=ALU.add)
                arin = pdram.tile([128, w], F32, tag=f"arin{l}{ci}")
                arout = pdram.tile([128, w], F32, tag=f"arout{l}{ci}")
                nc.sync.dma_start(arin[:], red[:])
                nc.gpsimd.collective_compute(
                    "AllReduce", ALU.add, replica_groups=RG,
                    ins=[arin.opt()], outs=[arout.opt()])
                arouts[(l, ci)] = (arout, w)

            def emit_ar_land(l, ci):
                arout, w = arouts[(l, ci)]
                g_t = pstat.tile([128, w], F32, tag=f"g{l}{ci}", name=f"g{l}{ci}")
                nc.sync.dma_start(g_t[:], arout[:])
                gchunk[(l, ci)] = g_t

            def _st(l, ci, tag, nm):
                return pstat.tile([128, nm], F32, tag=f"{tag}{l}{ci}",
                                  name=f"{tag}{l}{ci}")

            def emit_stats_pre(l, ci, do_sq, fastl):
                """DVE-only stats from the pre-reduced AR result (safe to emit
                mid-loop: no ACT ops to block later drains)."""
                g_t = gchunk[(l, ci)]
                bd = _bounds(l)
                nm = bd[ci + 1] - bd[ci]
                m1 = _st(l, ci, "m1", nm)
                nc.vector.tensor_scalar_mul(m1[:], g_t[:, 0:nm], 1.0 / B)
                if fastl and not do_sq:
                    negm = _st(l, ci, "negm", nm)
                    nc.vector.tensor_scalar_mul(negm[:], g_t[:, 0:nm], -1.0 / B)
                    stats[(l, ci)] = dict(m1=m1, negm=negm, fast=True)
                    return
                msq, m1sq, v = (_st(l, ci, x, nm) for x in ("msq", "m1sq", "v"))
                nc.vector.tensor_scalar_mul(msq[:], g_t[:, nm:2 * nm], 1.0 / B)
                nc.vector.tensor_tensor(m1sq[:], m1[:], m1[:], op=ALU.mult)
                nc.vector.tensor_tensor(v[:], msq[:], m1sq[:], op=ALU.subtract)
                nc.vector.tensor_scalar_add(v[:], v[:], EPS)
                stats[(l, ci)] = dict(m1=m1, v=v, fast=False)

            def emit_stats_post(l, ci, fastl):
                """ACT sqrt + downstream scale/bias (emit after the layer's
                drains so the ACT queue never blocks on a pending AR)."""
                d = stats[(l, ci)]
                if d["fast"]:
                    return
                bd = _bounds(l)
                c0 = bd[ci]
                nm = bd[ci + 1] - c0
                gcol = cons[f"g{l}"][:, c0:c0 + nm]
                btcol = cons[f"bt{l}"][:, c0:c0 + nm]
                m1, v = d["m1"], d["v"]
                r, rp, mt, cc = (_st(l, ci, x, nm) for x in ("r", "rp", "mt", "c"))
                sq = _st(l, ci, "sq", nm)
                nc.scalar.activation(sq[:], v[:], ACT.Sqrt)
                nc.vector.reciprocal(r[:], sq[:])
                nc.vector.tensor_tensor(rp[:], gcol, r[:], op=ALU.mult)
                nc.vector.tensor_tensor(mt[:], m1[:], rp[:], op=ALU.mult)
                nc.vector.tensor_tensor(cc[:], btcol, mt[:], op=ALU.subtract)
                d.update(rp=rp, c=cc)
                if l < 3:
                    gi, u, u2, tthr, s, s2, sneg = (
                        _st(l, ci, x, nm)
                        for x in ("gi", "u", "u2", "tthr", "s", "s2", "sneg"))
                    nc.vector.reciprocal(gi[:], gcol)
                    nc.vector.tensor_tensor(u[:], btcol, gi[:], op=ALU.mult)
                    nc.vector.tensor_tensor(u2[:], u[:], sq[:], op=ALU.mult)
                    nc.vector.tensor_tensor(tthr[:], m1[:], u2[:], op=ALU.subtract)
                    nc.scalar.activation(s[:], gcol, ACT.Sign)
                    nc.vector.tensor_scalar_mul(s2[:], s[:], 2.0)
                    nc.vector.tensor_scalar_mul(sneg[:], s[:], -1.0)
                    d.update(tthr=tthr, s2=s2, sneg=sneg)

            def chunk_of(l, k):
                bd = _bounds(l)
                for ci in range(len(bd) - 1):
                    if k < bd[ci + 1]:
                        return ci, k - bd[ci]

            def sign_wave(l, dst3, h_t, krange, dve_only=False):
                """a[:, k, :] = sign-of-bn for k in krange; alternate ACT/DVE.
                dve_only for AR-end-gated chunks: keeps the ACT queue free of
                AR-gated ops so the next layer's drains + wdr DMA triggers
                (which ride the ACT queue in-order) are never blocked."""
                for k in krange:
                    ci, j = chunk_of(l, k)
                    s = stats[(l, ci)]
                    hsl = h_t[:, k * BS:(k + 1) * BS]
                    dst = dst3[:, k, :]
                    if k % 2 == 1 and not dve_only:
                        scale = 1.0 if s["fast"] else s["rp"][:, j:j + 1]
                        bias = s["negm"][:, j:j + 1] if s["fast"] else s["c"][:, j:j + 1]
                        nc.scalar.activation(dst, hsl, ACT.Sign, bias=bias, scale=scale)
                    elif s["fast"]:
                        # {0,1} encoding: weights are +-2 with bias correction
                        nc.vector.tensor_scalar(out=dst, in0=hsl,
                                                scalar1=s["m1"][:, j:j + 1],
                                                scalar2=None, op0=ALU.is_ge)
                    else:
                        thr = s["tthr"][:, j:j + 1]
                        bt_ = pscr.tile([128, BS], F16, tag="scr", name=f"sgb_{l}_{k}")
                        nc.vector.tensor_scalar(out=bt_[:], in0=hsl, scalar1=thr,
                                                scalar2=None, op0=ALU.is_ge)
                        nc.vector.tensor_scalar(out=dst, in0=bt_[:],
                                                scalar1=s["s2"][:, j:j + 1],
                                                scalar2=s["sneg"][:, j:j + 1],
                                                op0=ALU.mult, op1=ALU.add)

            def drain(l, m, n, ps, h_t, do_sq):
                hs = h_t[:, m * BS + n * 512: m * BS + n * 512 + 512]
                col = 2 * m + n
                nc.scalar.activation(hs, ps[:], ACT.Identity,
                                     bias=cons[f"b{l}"][:, m:m + 1], scale=1.0,
                                     accum_out=parts[l][:, col:col + 1])
                if do_sq:
                    scr = pscr.tile([128, 512], F32, tag="scr", name=f"sq_{l}_{m}_{n}")
                    nc.scalar.activation(scr[:], hs, ACT.Square,
                                         accum_out=parts[l][:, 32 + col:32 + col + 1])

            def debug_out(src_ap, cast=False):
                if cast:
                    t = pscr.tile([128, BS], F32, tag="scr", name="dbgcast")
                    nc.vector.tensor_copy(t[:C, :], src_ap)
                    src_ap = t[:C, :]
                nc.sync.dma_start(out_d[:], src_ap)

            # ===================== Layer 1 =====================
            h1 = ph.tile([128, KH * BS], F32, tag="ph", name="h1")
            parts[1] = pstat.tile([128, 64], F32, tag="parts1", name="parts1")
            do_sq1 = not fast[0]
            bd1 = _bounds(1)

            def l1_mtile_alloc(m):
                st = {}
                st["wst"] = w1_pf.pop(m)
                st["w8lo"] = pw8.tile([128, KD * 128], F8E5, tag="w8",
                                      name=f"w8lo_{m}")
                st["w8lov"] = st["w8lo"][:].rearrange("p (k c) -> p k c", c=128)
                st["pss"] = [ppsum.tile([128, 512], F32, tag="ps",
                                        name=f"ps1_{m}_{n}") for n in range(NB)]
                return st

            def l1_chunk(st, c):
                wst, w8lo, w8lov, pss = (st["wst"], st["w8lo"], st["w8lov"],
                                         st["pss"])
                for k in range(c * KC, (c + 1) * KC):
                    lhsT = wst[:, k * 128:(k + 1) * 128]
                    for n in range(NB):
                        nc.tensor.matmul(
                            pss[n][:], lhsT,
                            xhi[:, k * BS + n * 512: k * BS + n * 512 + 512],
                            start=(k == 0), stop=False)
                nc.vector.tensor_scalar_mul(
                    w8lo[:, c * KC * 128:(c + 1) * KC * 128],
                    wst[:, c * KC * 128:(c + 1) * KC * 128], 2.0 ** -12)
                for t in range(c * KC // 2, (c + 1) * KC // 2):
                    lhsT = w8lov[:, 2 * t:2 * t + 2, :]
                    for n in range(NB):
                        nc.tensor.matmul(
                            pss[n][:], lhsT,
                            xlo8v[:, 2 * t:2 * t + 2, n * 512:n * 512 + 512],
                            start=False, stop=(t == KD // 2 - 1), perf_mode=DRM)

            def l1_tail(m, st):
                # W1 prefetch 3 ahead; Wdr gens 3..7 during m=8..12
                if m + 3 < KH and m + 3 not in w1_pf:
                    w1_pf[m + 3] = pw.tile([128, KD * 128], F8E4, tag="w",
                                           name=f"w1_{m + 3}")
                    nc.scalar.dma_start(
                        w1_pf[m + 3][:],
                        w1_d[:, (m + 3) * KD * 128:(m + 4) * KD * 128])
                if 8 <= m <= 12:
                    emit_wdr(m - 5)
                if m in (1, 3, 5, 7, 9, 11):
                    warm_ar()
                for n in range(NB):
                    drain(1, m, n, st["pss"][n], h1, do_sq1)
                for ci in range(len(bd1) - 1):
                    if m == bd1[ci + 1] - 1:
                        emit_ar_fire(1, ci, do_sq1)
                        if ci > 0:
                            emit_ar_land(1, ci - 1)

            # m=0,1 interleaved per x chunk: the PE rides the incoming DMA wave
            st01 = {m: l1_mtile_alloc(m) for m in (0, 1)}
            for c in range(XCH):
                for m in (0, 1):
                    l1_chunk(st01[m], c)
            for m in (0, 1):
                l1_tail(m, st01[m])
            for m in range(2, KH):
                st = l1_mtile_alloc(m)
                for c in range(XCH):
                    l1_chunk(st, c)
                l1_tail(m, st)
            emit_ar_land(1, len(bd1) - 2)

            if stage == 1:
                debug_out(h1[:C, :BS])

            a2 = pa.tile([128, KH, BS], F8E4, tag="pa", name="a2")
            nch1 = len(bd1) - 1
            for ci in range(nch1):
                emit_stats_pre(1, ci, do_sq1, fast[0])
                emit_stats_post(1, ci, fast[0])
                sign_wave(1, a2, h1, range(bd1[ci], bd1[ci + 1]),
                          dve_only=(ci == nch1 - 1))
            if stage == 2:
                debug_out(a2[:C, 0, :], cast=True)

            # ===================== Layers 2, 3 =====================
            def dense_dr(l, a_in):
                h_t = ph.tile([128, KH * BS], F32, tag="ph", name=f"h{l}")
                parts[l] = pstat.tile([128, 64], F32, tag=f"parts{l}", name=f"parts{l}")
                do_sq = (l == 3) or not fast[l - 1]
                bd = _bounds(l)
                # t-phases matching the PRODUCING layer's sign chunks
                pb_in = _bounds(l - 1)
                tph = [(pb_in[i] // 2, pb_in[i + 1] // 2) for i in range(len(pb_in) - 1)]
                for q in range(KH // 4):
                    ms = range(4 * q, 4 * q + 4)
                    pss = {m: [ppsum.tile([128, 512], F32, tag="ps",
                                          name=f"ps{l}_{m}_{n}") for n in range(NB)]
                           for m in ms}
                    w8 = {m: wdr_pf.pop((l, m)) for m in ms}
                    for ta, tb in tph:
                        for m in ms:
                            w8v = w8[m][:].rearrange("p (k c) -> p k c", c=128)
                            for t in range(ta, tb):
                                lhsT = w8v[:, 2 * t:2 * t + 2, :]
                                for n in range(NB):
                                    nc.tensor.matmul(
                                        pss[m][n][:], lhsT,
                                        a_in[:, 2 * t:2 * t + 2, n * 512:n * 512 + 512],
                                        start=(t == 0), stop=(t == KH // 2 - 1),
                                        perf_mode=DRM)
                    for m in ms:
                        gen = (l - 2) * 16 + m + 8
                        if gen < 32:
                            emit_wdr(gen)
                        if m in (1, 5):
                            warm_ar()
                        for n in range(NB):
                            drain(l, m, n, pss[m][n], h_t, do_sq)
                        for ci in range(len(bd) - 1):
                            if m == bd[ci + 1] - 1:
                                emit_ar_fire(l, ci, do_sq)
                                if ci > 0:
                                    emit_ar_land(l, ci - 1)
                emit_ar_land(l, len(bd) - 2)
                return h_t

            if stage >= 3:
                h2 = dense_dr(2, a2[:])
                a3 = pb.tile([128, KH, BS], F8E4, tag="pb", name="a3")
                bd2 = _bounds(2)
                for ci in range(len(bd2) - 1):
                    emit_stats_pre(2, ci, not fast[1], fast[1])
                    emit_stats_post(2, ci, fast[1])
                    sign_wave(2, a3, h2, range(bd2[ci], bd2[ci + 1]),
                              dve_only=(ci >= 1))
                if stage == 3:
                    debug_out(a3[:C, 0, :], cast=True)

            if stage >= 4:
                h3 = dense_dr(3, a3[:])
                # y3 = clip(bn3(h3), -1, 1) in fp16; L4 matmuls follow per k
                logits = plog.tile([16, BS], F32, tag="logits")
                ps4 = [ppsum.tile([128, 512], F32, tag="ps", name=f"ps4_{n}")
                       for n in range(NB)]
                y3dbg = None
                bd3 = _bounds(3)
                for ci in range(len(bd3) - 1):
                    emit_stats_pre(3, ci, True, False)
                    emit_stats_post(3, ci, False)
                    s = stats[(3, ci)]
                    for k in range(bd3[ci], bd3[ci + 1]):
                        j = k - bd3[ci]
                        scr = pscr.tile([128, BS], F32, tag="scr", name=f"y3s_{k}")
                        nc.scalar.activation(scr[:], h3[:, k * BS:(k + 1) * BS],
                                             ACT.Identity, bias=s["c"][:, j:j + 1],
                                             scale=s["rp"][:, j:j + 1])
                        y3k = py3.tile([128, BS], F16, tag="y3", name=f"y3_{k}")
                        nc.vector.tensor_scalar(out=y3k[:], in0=scr[:],
                                                scalar1=-1.0, scalar2=1.0,
                                                op0=ALU.max, op1=ALU.min)
                        if k == 0:
                            y3dbg = y3k
                        if stage >= 5:
                            for n in range(NB):
                                nc.tensor.matmul(
                                    ps4[n][:C, :], w4f[:, k * C:(k + 1) * C],
                                    y3k[:, n * 512:(n + 1) * 512],
                                    start=(k == 0), stop=(k == KH - 1))
                if stage == 4:
                    debug_out(y3dbg[:C, :], cast=True)

            if stage >= 5:
                # ===== logits + log-softmax, 4 chunks of 256 cols =====
                for qq in range(4):
                    bank = ps4[qq // 2]
                    bsl = slice((qq % 2) * 256, (qq % 2) * 256 + 256)
                    qsl = slice(qq * 256, (qq + 1) * 256)
                    # logits on DVE (PSUM read) in parallel with exp on ACT
                    nc.vector.tensor_scalar(out=logits[:C, qsl], in0=bank[:C, bsl],
                                            scalar1=b4s[:C, :], scalar2=None,
                                            op0=ALU.add)
                    e_q = ptail.tile([16, 256], F32, tag="tl", name=f"e_{qq}")
                    nc.scalar.activation(e_q[:C, :], bank[:C, bsl], ACT.Exp,
                                         bias=b4s[:C, :], scale=1.0)
                    ps5 = ppsum.tile([128, 256], F32, tag="ps", name=f"ps5_{qq}")
                    nc.tensor.matmul(ps5[:1, :], ones10[:C, :], e_q[:C, :],
                                     start=True, stop=True)
                    lse_q = ptail.tile([16, 256], F32, tag="tl", name=f"lse_{qq}")
                    nc.scalar.activation(lse_q[:1, :], ps5[:1, :], ACT.Ln)
                    ps6 = ppsum.tile([128, 256], F32, tag="ps", name=f"ps6_{qq}")
                    nc.tensor.matmul(ps6[:C, :], onesC[:1, :C], lse_q[:1, :],
                                     start=True, stop=True)
                    outs_q = ptail.tile([16, 256], F32, tag="tl", name=f"o_{qq}")
                    nc.vector.tensor_tensor(outs_q[:C, :], logits[:C, qsl],
                                            ps6[:C, :], op=ALU.subtract)
                    if stage >= 6:
                        nc.sync.dma_start(out_d[:, qsl], outs_q[:C, :])
                if stage == 5:
                    debug_out(logits[:C, :])

    nc.compile()
    return nc


def _prep_inputs(x, W1, b1, g1, bt1, W2, b2, g2, bt2, W3, b3, g3, bt3, W4, b4):
    """Host-side sharding + layout prep (sign, fp8 cast, p-major packing)."""
    def as32(a):
        return np.ascontiguousarray(np.asarray(a, dtype=np.float32))

    f8 = ml_dtypes.float8_e4m3

    def sgn(w):
        return np.where(np.asarray(w) >= 0, np.float32(1.0), np.float32(-1.0))

    def pack_w(w, kt, uks=()):
        # [H_out, K] -> [128, (H_out/128) * K] with per-m-tile p-major blocks.
        # uks: k-blocks whose activations come {0,1}-encoded -> weights +-2.
        s = sgn(w).reshape(-1, 128, kt, 128)            # [m, c, k, p]
        if uks:
            s[:, :, sorted(uks), :] *= 2.0
        s = s.transpose(0, 3, 2, 1).reshape(s.shape[0], 128, kt * 128)
        return np.ascontiguousarray(
            s.transpose(1, 0, 2).reshape(128, -1)).astype(f8)

    def ok(g, bt):
        g, bt = np.asarray(g), np.asarray(bt)
        return bool(not np.any(bt) and np.all(g > 0))

    def ucorr(w, uks):
        # bias correction: -sum over u-encoded k-blocks of sign(w)
        if not uks:
            return 0.0
        s = sgn(w).reshape(w.shape[0], -1, 128)
        return s[:, sorted(uks), :].sum(axis=(1, 2))

    x = as32(x)
    u2 = _u_ks(1) if ok(g1, bt1) else set()
    u3 = _u_ks(2) if ok(g2, bt2) else set()
    W2, W3, b2, b3 = as32(W2), as32(W3), as32(b2), as32(b3)
    shared = {
        "w1pk": pack_w(as32(W1), KD),
        "w2pk": pack_w(W2, KH, u2),
        "w3pk": pack_w(W3, KH, u3),
    }
    b2 = b2 - ucorr(W2, u2)
    b3 = b3 - ucorr(W3, u3)
    cvecs = (b1, g1, bt1, b2, g2, bt2, b3, g3, bt3)
    cpk = np.empty((128, KH * len(cvecs)), np.float32)
    for i, v in enumerate(cvecs):
        cpk[:, i * KH:(i + 1) * KH] = as32(v).reshape(KH, 128).T
    shared["cpk"] = cpk
    w4T = np.ascontiguousarray(as32(W4).T)          # [H, C]
    w4pk = np.empty((128, C * KH), np.float16)
    for k in range(KH):
        w4pk[:, k * C:(k + 1) * C] = w4T[k * 128:(k + 1) * 128, :].astype(np.float16)
    shared["w4pk"] = w4pk
    b4p = np.zeros((16, 1), np.float32)
    b4p[:C, 0] = as32(b4).reshape(-1)
    shared["c_b4"] = b4p

    in_maps = []
    for cr in range(NCORES):
        xT = np.ascontiguousarray(x[cr * BS:(cr + 1) * BS].T)     # [D, BS]
        hi = xT.astype(np.float16)
        lo8 = ((xT - hi.astype(np.float32)) * 4096.0).astype(f8)
        # p-major pack: [D, BS] -> [128, KD*BS]
        hi_pk = np.ascontiguousarray(
            hi.reshape(KD, 128, BS).transpose(1, 0, 2).reshape(128, KD * BS))
        lo_pk = np.ascontiguousarray(
            lo8.reshape(KD, 128, BS).transpose(1, 0, 2).reshape(128, KD * BS))
        m = dict(shared)
        m["xT_hi"] = hi_pk
        m["xT_lo8"] = lo_pk
        in_maps.append(m)
    return in_maps


def _fast_flags(inputs):
    """Mean-only BN boundary valid when beta==0 and gamma>0."""
    def ok(g, bt):
        g, bt = np.asarray(g), np.asarray(bt)
        return bool(not np.any(bt) and np.all(g > 0))

    return (ok(inputs["g1"], inputs["bt1"]), ok(inputs["g2"], inputs["bt2"]))


def kernel(**inputs) -> np.ndarray:
    from concourse.bass_utils import run_bass_kernel_spmd

    fast = _fast_flags(inputs)
    if _CACHE.get("fast") != fast:
        _CACHE["nc"] = _build(fast=fast)
        _CACHE["fast"] = fast
    nc = _CACHE["nc"]
    in_maps = _prep_inputs(**inputs)
    res = run_bass_kernel_spmd(nc, in_maps, list(range(NCORES)))
    out = np.concatenate([res.results[c]["outT"].T for c in range(NCORES)], axis=0)
    return out.astype(np.float32)


# revision 15
# speedup vs baseline: 1.0101x; 1.0101x over previous
"""Trainium2 Bass kernel for nn_BinarizedCifar10MLP — v3.

Data-parallel over batch (8192/8 = 1024 rows/core), feature-major layout.

vs v2 (573us):
  - All weight signing moved to the HOST: W1 ships as fp8e4 +-1 (6.3MB,
    was 12.6MB bf16 + on-device sign), W2/W3 ship as fp8e4 +-1 in DR
    layout (no bf16 read + sign + DRAM round-trip prepass at all).
  - DMA queue discipline: bulk loads (x, W1, Wdr) ride the ACT hwdge
    queue; the sync queue carries only AR traffic + consts + final out.
    W1 m0 is FIRST on the queue (v2 had it behind all 9.4MB of x -> 43us
    PE stall at start); x chunks interleave with W1 m0 sub-tiles and the
    L1 m-loop consumes x chunk-by-chunk, so the PE rides the DMA wave.
  - 3-chunk BN-stat AllReduce for L2/L3 (m 0-9 / 10-13 / 14-15): AR-A
    fires at ~60% of the (short) layer instead of 87%, landing before
    the layer ends; quad-grouped phase-major matmul emission gives the
    PE a 4-m-tile runway on already-signed k-tiles while the tail AR
    lands. Stat sums are n-pair-reduced BEFORE the AR (half payload).
  - log-softmax tail in 4 chunks of 256 cols, exp computed straight
    from PSUM in parallel with the logits drain (DVE reads PSUM).
"""

import sys

sys.path.insert(0, "/opt/trn_rl_repo")

import numpy as np
import ml_dtypes

B, D, H, C = 8192, 3 * 32 * 32, 2048, 10
EPS = 1e-5
NCORES = 8
BS = B // NCORES          # 1024 batch rows per core
KD = D // 128             # 24 k-tiles over input dim
KH = H // 128             # 16 k-tiles over hidden dim
NB = BS // 512            # 2 free-dim chunks of 512
CHK = {1: (14, 16), 2: (10, 14, 16), 3: (10, 14, 16)}  # AR chunk end bounds
XCH = 4                   # x DMA chunks (6 k-tiles each)
KC = KD // XCH            # 6 k-tiles per x chunk

_CACHE = {}


def _bounds(l):
    return (0,) + CHK[l]


def _u_ks(l_prod):
    """k-tiles of the layer-l_prod sign output that are {0,1}-encoded (single
    DVE is_ge op); the consumer weights are host-scaled to +-2 on those blocks
    with the -rowsum(sign W) correction folded into the bias."""
    bd = _bounds(l_prod)
    nch = len(bd) - 1
    dve = {nch - 1} if l_prod == 1 else set(range(1, nch))
    ks = set()
    for ci in range(nch):
        for k in range(bd[ci], bd[ci + 1]):
            if ci in dve or k % 2 == 0:
                ks.add(k)
    return ks


def _build(stage=7, fast=(False, False)):
    import concourse.bacc as bacc
    import concourse.mybir as mybir
    import concourse.tile as tile

    F32 = mybir.dt.float32
    F16 = mybir.dt.float16
    F8E4 = mybir.dt.float8e4
    F8E5 = mybir.dt.float8e5
    DRM = mybir.MatmulPerfMode.DoubleRow
    ACT = mybir.ActivationFunctionType
    ALU = mybir.AluOpType
    RG = [list(range(NCORES))]

    nc = bacc.Bacc("TRN2", target_bir_lowering=False, debug=False, num_devices=NCORES)

    # ---- I/O ----
    xhi_d = nc.dram_tensor("xT_hi", [128, KD * BS], F16, kind="ExternalInput").ap()
    xlo_d = nc.dram_tensor("xT_lo8", [128, KD * BS], F8E4, kind="ExternalInput").ap()
    w1_d = nc.dram_tensor("w1pk", [128, KH * KD * 128], F8E4, kind="ExternalInput").ap()
    w2_d = nc.dram_tensor("w2pk", [128, KH * KH * 128], F8E4, kind="ExternalInput").ap()
    w3_d = nc.dram_tensor("w3pk", [128, KH * KH * 128], F8E4, kind="ExternalInput").ap()
    CNAMES = ("b1", "g1", "bt1", "b2", "g2", "bt2", "b3", "g3", "bt3")
    cpk_d = nc.dram_tensor("cpk", [128, KH * len(CNAMES)], F32, kind="ExternalInput").ap()
    w4pk_d = nc.dram_tensor("w4pk", [128, C * KH], F16, kind="ExternalInput").ap()
    b4_d = nc.dram_tensor("c_b4", [16, 1], F32, kind="ExternalInput").ap()
    out_d = nc.dram_tensor("outT", [C, BS], F32, kind="ExternalOutput").ap()

    wl_d = {2: w2_d, 3: w3_d}

    with tile.TileContext(nc) as tc:
        with (
            tc.tile_pool(name="pconst", bufs=1) as pconst,
            tc.tile_pool(name="pstat", bufs=1) as pstat,
            tc.tile_pool(name="plog", bufs=1) as plog,
            tc.tile_pool(name="ptail", bufs=6) as ptail,
            tc.tile_pool(name="pscr", bufs=3) as pscr,
            tc.tile_pool(name="pw", bufs=4) as pw,
            tc.tile_pool(name="pw8", bufs=2) as pw8,
            tc.tile_pool(name="pwdr", bufs=8) as pwdr,
            tc.tile_pool(name="py3", bufs=4) as py3,
            tc.tile_pool(name="ph", bufs=1) as ph,
            tc.tile_pool(name="pa", bufs=1) as pa,
            tc.tile_pool(name="pb", bufs=1) as pb,
            tc.tile_pool(name="ppsum", bufs=8, space="PSUM") as ppsum,
            tc.tile_pool(name="pdram", bufs=16, space="DRAM") as pdram,
        ):
            # ---- warmup AllReduce: absorbs ncfw first-collective staging ----
            wuin = pdram.tile([128, 4], F32, tag="wuin")
            wuout = pdram.tile([128, 4], F32, tag="wuout")
            wusrc = pstat.tile([128, 4], F32, tag="wusrc")
            nc.vector.memset(wusrc[:], 0.0)
            nc.sync.dma_start(wuin[:], wusrc[:])
            nc.gpsimd.collective_compute(
                "AllReduce", ALU.add, replica_groups=RG,
                ins=[wuin.opt()], outs=[wuout.opt()])

            wuout1 = pdram.tile([128, 4], F32, tag="wuout1")

            def warm_ar(l, m):
                # keep the CC stream hot (cold-picked collectives take ~30us
                # vs ~9us warm); gated on parts col m so it fires mid-layer
                lo = max(0, m - 3)
                wi = pdram.tile([128, 4], F32, tag=f"wi{l}_{m}")
                nc.sync.dma_start(wi[:], parts[l][:, lo:lo + 4])
                nc.gpsimd.collective_compute(
                    "AllReduce", ALU.add, replica_groups=RG,
                    ins=[wi.opt()], outs=[wuout1.opt()])

            # ---- constants (sync queue; small, land early) ----
            cpk = pconst.tile([128, KH * len(CNAMES)], F32, tag="cpk")
            nc.sync.dma_start(cpk[:], cpk_d)
            cons = {name: cpk[:, i * KH:(i + 1) * KH] for i, name in enumerate(CNAMES)}
            b4s = pconst.tile([16, 1], F32, tag="b4")
            nc.sync.dma_start(b4s[:], b4_d)
            ones10 = pconst.tile([16, 1], F32, tag="ones10")
            nc.vector.memset(ones10[:], 1.0)
            onesC = pconst.tile([1, 16], F32, tag="onesC")
            nc.vector.memset(onesC[:], 1.0)

            # ---- bulk loads: scalar (ACT hwdge) queue ----
            # W1 m0 interleaved chunk-wise with x so the PE starts ~5us in.
            xhi = pa.tile([128, KD * BS], F16, tag="pa", name="xhi")
            xlo8 = pb.tile([128, KD * BS], F8E4, tag="pb", name="xlo8")
            w1_pf = {}
            for m in range(3):
                w1_pf[m] = pw.tile([128, KD * 128], F8E4, tag="w", name=f"w1_{m}")
            for c in range(XCH):
                nc.scalar.dma_start(
                    w1_pf[0][:, c * KC * 128:(c + 1) * KC * 128],
                    w1_d[:, c * KC * 128:(c + 1) * KC * 128])
                sl = slice(c * KC * BS, (c + 1) * KC * BS)
                nc.scalar.dma_start(xhi[:, sl], xhi_d[:, sl])
                nc.scalar.dma_start(xlo8[:, sl], xlo_d[:, sl])
            for m in range(1, 3):
                nc.scalar.dma_start(
                    w1_pf[m][:], w1_d[:, m * KD * 128:(m + 1) * KD * 128])
            xlo8v = xlo8[:].rearrange("p (k c) -> p k c", c=BS)

            # Wdr stream: fp8 +-1 DR-layout weights for L2/L3, 8-deep ring.
            # gens 0..15 = L2 m0..15, 16..31 = L3 m0..15.
            wdr_pf = {}

            def emit_wdr(gen):
                l, m = (2, gen) if gen < 16 else (3, gen - 16)
                w8t = pwdr.tile([128, KH * 128], F8E4, tag="wdr", name=f"wdr_{l}_{m}")
                nc.scalar.dma_start(w8t[:], wl_d[l][:, m * 2048:(m + 1) * 2048])
                wdr_pf[(l, m)] = w8t

            for gen in range(3):
                emit_wdr(gen)

            w4f = pconst.tile([128, C * KH], F16, tag="w4f")
            nc.scalar.dma_start(w4f[:], w4pk_d)

            parts = {}
            gchunk = {}     # (l, ci) -> allreduced pre-reduced stats tile
            stats = {}      # (l, ci) -> dict of stat tiles
            arouts = {}

            def emit_ar_fire(l, ci, do_sq):
                """parts chunk cols -> DRAM -> AllReduce."""
                bd = _bounds(l)
                c0, c1 = bd[ci], bd[ci + 1]
                nm = c1 - c0
                w = 2 * nm if do_sq else nm
                arin = pdram.tile([128, w], F32, tag=f"arin{l}{ci}")
                arout = pdram.tile([128, w], F32, tag=f"arout{l}{ci}")
                nc.sync.dma_start(arin[:, 0:nm], parts[l][:, c0:c1])
                if do_sq:
                    nc.sync.dma_start(arin[:, nm:w], parts[l][:, 16 + c0:16 + c1])
                nc.gpsimd.collective_compute(
                    "AllReduce", ALU.add, replica_groups=RG,
                    ins=[arin.opt()], outs=[arout.opt()])
                arouts[(l, ci)] = (arout, w)

            def emit_ar_land(l, ci):
                arout, w = arouts[(l, ci)]
                g_t = pstat.tile([128, w], F32, tag=f"g{l}{ci}", name=f"g{l}{ci}")
                nc.sync.dma_start(g_t[:], arout[:])
                gchunk[(l, ci)] = g_t

            def _st(l, ci, tag, nm):
                return pstat.tile([128, nm], F32, tag=f"{tag}{l}{ci}",
                                  name=f"{tag}{l}{ci}")

            def emit_stats_pre(l, ci, do_sq, fastl):
                """DVE-only stats from the pre-reduced AR result (safe to emit
                mid-loop: no ACT ops to block later drains)."""
                g_t = gchunk[(l, ci)]
                bd = _bounds(l)
                nm = bd[ci + 1] - bd[ci]
                m1 = _st(l, ci, "m1", nm)
                nc.vector.tensor_scalar_mul(m1[:], g_t[:, 0:nm], 1.0 / B)
                if fastl and not do_sq:
                    negm = _st(l, ci, "negm", nm)
                    nc.vector.tensor_scalar_mul(negm[:], g_t[:, 0:nm], -1.0 / B)
                    stats[(l, ci)] = dict(m1=m1, negm=negm, fast=True)
                    return
                msq, m1sq, v = (_st(l, ci, x, nm) for x in ("msq", "m1sq", "v"))
                nc.vector.tensor_scalar_mul(msq[:], g_t[:, nm:2 * nm], 1.0 / B)
                nc.vector.tensor_tensor(m1sq[:], m1[:], m1[:], op=ALU.mult)
                nc.vector.tensor_tensor(v[:], msq[:], m1sq[:], op=ALU.subtract)
                nc.vector.tensor_scalar_add(v[:], v[:], EPS)
                stats[(l, ci)] = dict(m1=m1, v=v, fast=False)

            def emit_stats_post(l, ci, fastl):
                """ACT sqrt + downstream scale/bias (emit after the layer's
                drains so the ACT queue never blocks on a pending AR)."""
                d = stats[(l, ci)]
                if d["fast"]:
                    return
                bd = _bounds(l)
                c0 = bd[ci]
                nm = bd[ci + 1] - c0
                gcol = cons[f"g{l}"][:, c0:c0 + nm]
                btcol = cons[f"bt{l}"][:, c0:c0 + nm]
                m1, v = d["m1"], d["v"]
                r, rp, mt, cc = (_st(l, ci, x, nm) for x in ("r", "rp", "mt", "c"))
                sq = _st(l, ci, "sq", nm)
                nc.scalar.activation(sq[:], v[:], ACT.Sqrt)
                nc.vector.reciprocal(r[:], sq[:])
                nc.vector.tensor_tensor(rp[:], gcol, r[:], op=ALU.mult)
                nc.vector.tensor_tensor(mt[:], m1[:], rp[:], op=ALU.mult)
                nc.vector.tensor_tensor(cc[:], btcol, mt[:], op=ALU.subtract)
                d.update(rp=rp, c=cc)
                if l < 3:
                    gi, u, u2, tthr, s, s2, sneg = (
                        _st(l, ci, x, nm)
                        for x in ("gi", "u", "u2", "tthr", "s", "s2", "sneg"))
                    nc.vector.reciprocal(gi[:], gcol)
                    nc.vector.tensor_tensor(u[:], btcol, gi[:], op=ALU.mult)
                    nc.vector.tensor_tensor(u2[:], u[:], sq[:], op=ALU.mult)
                    nc.vector.tensor_tensor(tthr[:], m1[:], u2[:], op=ALU.subtract)
                    nc.scalar.activation(s[:], gcol, ACT.Sign)
                    nc.vector.tensor_scalar_mul(s2[:], s[:], 2.0)
                    nc.vector.tensor_scalar_mul(sneg[:], s[:], -1.0)
                    d.update(tthr=tthr, s2=s2, sneg=sneg)

            def chunk_of(l, k):
                bd = _bounds(l)
                for ci in range(len(bd) - 1):
                    if k < bd[ci + 1]:
                        return ci, k - bd[ci]

            def sign_wave(l, dst3, h_t, krange, dve_only=False):
                """a[:, k, :] = sign-of-bn for k in krange; alternate ACT/DVE.
                dve_only for AR-end-gated chunks: keeps the ACT queue free of
                AR-gated ops so the next layer's drains + wdr DMA triggers
                (which ride the ACT queue in-order) are never blocked."""
                for k in krange:
                    ci, j = chunk_of(l, k)
                    s = stats[(l, ci)]
                    hsl = h_t[:, k * BS:(k + 1) * BS]
                    dst = dst3[:, k, :]
                    if k % 2 == 1 and not dve_only:
                        scale = 1.0 if s["fast"] else s["rp"][:, j:j + 1]
                        bias = s["negm"][:, j:j + 1] if s["fast"] else s["c"][:, j:j + 1]
                        nc.scalar.activation(dst, hsl, ACT.Sign, bias=bias, scale=scale)
                    elif s["fast"]:
                        # {0,1} encoding: weights are +-2 with bias correction
                        nc.vector.tensor_scalar(out=dst, in0=hsl,
                                                scalar1=s["m1"][:, j:j + 1],
                                                scalar2=None, op0=ALU.is_ge)
                    else:
                        thr = s["tthr"][:, j:j + 1]
                        bt_ = pscr.tile([128, BS], F16, tag="scr", name=f"sgb_{l}_{k}")
                        nc.vector.tensor_scalar(out=bt_[:], in0=hsl, scalar1=thr,
                                                scalar2=None, op0=ALU.is_ge)
                        nc.vector.tensor_scalar(out=dst, in0=bt_[:],
                                                scalar1=s["s2"][:, j:j + 1],
                                                scalar2=s["sneg"][:, j:j + 1],
                                                op0=ALU.mult, op1=ALU.add)

            def drain(l, m, n, ps, h_t, do_sq):
                hs = h_t[:, m * BS + n * 512: m * BS + n * 512 + 512]
                nc.scalar.activation(hs, ps[:], ACT.Identity,
                                     bias=cons[f"b{l}"][:, m:m + 1], scale=1.0)

            def msum(l, m, h_t, do_sq):
                """Per-m-tile batch sum (+sumsq) on DVE: keeps the ACT queue
                clear of accumulator reads and Square passes."""
                hrow = h_t[:, m * BS:(m + 1) * BS]
                nc.vector.tensor_reduce(
                    parts[l][:, m:m + 1],
                    hrow.rearrange("p (a b) -> p a b", a=1),
                    axis=mybir.AxisListType.X, op=ALU.add)
                if do_sq:
                    scr = pscr.tile([128, BS], F32, tag="scr", name=f"sq_{l}_{m}")
                    nc.vector.tensor_tensor(scr[:], hrow, hrow, op=ALU.mult)
                    nc.vector.tensor_reduce(
                        parts[l][:, 16 + m:16 + m + 1],
                        scr[:].rearrange("p (a b) -> p a b", a=1),
                        axis=mybir.AxisListType.X, op=ALU.add)

            def debug_out(src_ap, cast=False):
                if cast:
                    t = pscr.tile([128, BS], F32, tag="scr", name="dbgcast")
                    nc.vector.tensor_copy(t[:C, :], src_ap)
                    src_ap = t[:C, :]
                nc.sync.dma_start(out_d[:], src_ap)

            # ===================== Layer 1 =====================
            h1 = ph.tile([128, KH * BS], F32, tag="ph", name="h1")
            parts[1] = pstat.tile([128, 64], F32, tag="parts1", name="parts1")
            do_sq1 = not fast[0]
            bd1 = _bounds(1)

            def l1_mtile_alloc(m):
                st = {}
                st["wst"] = w1_pf.pop(m)
                st["w8lo"] = pw8.tile([128, KD * 128], F8E5, tag="w8",
                                      name=f"w8lo_{m}")
                st["w8lov"] = st["w8lo"][:].rearrange("p (k c) -> p k c", c=128)
                st["pss"] = [ppsum.tile([128, 512], F32, tag="ps",
                                        name=f"ps1_{m}_{n}") for n in range(NB)]
                return st

            def l1_chunk(st, c):
                wst, w8lo, w8lov, pss = (st["wst"], st["w8lo"], st["w8lov"],
                                         st["pss"])
                for k in range(c * KC, (c + 1) * KC):
                    lhsT = wst[:, k * 128:(k + 1) * 128]
                    for n in range(NB):
                        nc.tensor.matmul(
                            pss[n][:], lhsT,
                            xhi[:, k * BS + n * 512: k * BS + n * 512 + 512],
                            start=(k == 0), stop=False)
                nc.vector.tensor_scalar_mul(
                    w8lo[:, c * KC * 128:(c + 1) * KC * 128],
                    wst[:, c * KC * 128:(c + 1) * KC * 128], 2.0 ** -12)
                for t in range(c * KC // 2, (c + 1) * KC // 2):
                    lhsT = w8lov[:, 2 * t:2 * t + 2, :]
                    for n in range(NB):
                        nc.tensor.matmul(
                            pss[n][:], lhsT,
                            xlo8v[:, 2 * t:2 * t + 2, n * 512:n * 512 + 512],
                            start=False, stop=(t == KD // 2 - 1), perf_mode=DRM)

            def l1_tail(m, st):
                # W1 prefetch 3 ahead; Wdr gens 3..7 during m=8..12
                if m + 3 < KH and m + 3 not in w1_pf:
                    w1_pf[m + 3] = pw.tile([128, KD * 128], F8E4, tag="w",
                                           name=f"w1_{m + 3}")
                    nc.scalar.dma_start(
                        w1_pf[m + 3][:],
                        w1_d[:, (m + 3) * KD * 128:(m + 4) * KD * 128])
                if 8 <= m <= 12:
                    emit_wdr(m - 5)
                for n in range(NB):
                    drain(1, m, n, st["pss"][n], h1, do_sq1)
                msum(1, m, h1, do_sq1)
                if m in (8, 10, 12):
                    warm_ar(1, m)
                for ci in range(len(bd1) - 1):
                    if m == bd1[ci + 1] - 1:
                        emit_ar_fire(1, ci, do_sq1)
                        if ci > 0:
                            emit_ar_land(1, ci - 1)

            # m=0,1 interleaved per x chunk: the PE rides the incoming DMA wave
            st01 = {m: l1_mtile_alloc(m) for m in (0, 1)}
            for c in range(XCH):
                for m in (0, 1):
                    l1_chunk(st01[m], c)
            for m in (0, 1):
                l1_tail(m, st01[m])
            for m in range(2, KH):
                st = l1_mtile_alloc(m)
                for c in range(XCH):
                    l1_chunk(st, c)
                l1_tail(m, st)
            emit_ar_land(1, len(bd1) - 2)

            if stage == 1:
                debug_out(h1[:C, :BS])

            a2 = pa.tile([128, KH, BS], F8E4, tag="pa", name="a2")
            nch1 = len(bd1) - 1
            for ci in range(nch1):
                emit_stats_pre(1, ci, do_sq1, fast[0])
                emit_stats_post(1, ci, fast[0])
                sign_wave(1, a2, h1, range(bd1[ci], bd1[ci + 1]),
                          dve_only=(ci == nch1 - 1))
            if stage == 2:
                debug_out(a2[:C, 0, :], cast=True)

            # ===================== Layers 2, 3 =====================
            def dense_dr(l, a_in):
                h_t = ph.tile([128, KH * BS], F32, tag="ph", name=f"h{l}")
                parts[l] = pstat.tile([128, 64], F32, tag=f"parts{l}", name=f"parts{l}")
                do_sq = (l == 3) or not fast[l - 1]
                bd = _bounds(l)
                # t-phases matching the PRODUCING layer's sign chunks
                pb_in = _bounds(l - 1)
                tph = [(pb_in[i] // 2, pb_in[i + 1] // 2) for i in range(len(pb_in) - 1)]
                for q in range(KH // 4):
                    ms = range(4 * q, 4 * q + 4)
                    pss = {m: [ppsum.tile([128, 512], F32, tag="ps",
                                          name=f"ps{l}_{m}_{n}") for n in range(NB)]
                           for m in ms}
                    w8 = {m: wdr_pf.pop((l, m)) for m in ms}
                    for ta, tb in tph:
                        for m in ms:
                            w8v = w8[m][:].rearrange("p (k c) -> p k c", c=128)
                            for t in range(ta, tb):
                                lhsT = w8v[:, 2 * t:2 * t + 2, :]
                                for n in range(NB):
                                    nc.tensor.matmul(
                                        pss[m][n][:], lhsT,
                                        a_in[:, 2 * t:2 * t + 2, n * 512:n * 512 + 512],
                                        start=(t == 0), stop=(t == KH // 2 - 1),
                                        perf_mode=DRM)
                    for m in ms:
                        gen = (l - 2) * 16 + m + 8
                        if gen < 32:
                            emit_wdr(gen)
                        for n in range(NB):
                            drain(l, m, n, pss[m][n], h_t, do_sq)
                        msum(l, m, h_t, do_sq)
                        if m in (1, 5):
                            warm_ar(l, m)
                        for ci in range(len(bd) - 1):
                            if m == bd[ci + 1] - 1:
                                emit_ar_fire(l, ci, do_sq)
                                if ci > 0:
                                    emit_ar_land(l, ci - 1)
                emit_ar_land(l, len(bd) - 2)
                return h_t

            if stage >= 3:
                h2 = dense_dr(2, a2[:])
                a3 = pb.tile([128, KH, BS], F8E4, tag="pb", name="a3")
                bd2 = _bounds(2)
                for ci in range(len(bd2) - 1):
                    emit_stats_pre(2, ci, not fast[1], fast[1])
                    emit_stats_post(2, ci, fast[1])
                    sign_wave(2, a3, h2, range(bd2[ci], bd2[ci + 1]),
                              dve_only=(ci >= 1))
                if stage == 3:
                    debug_out(a3[:C, 0, :], cast=True)

            if stage >= 4:
                h3 = dense_dr(3, a3[:])
                # y3 = clip(bn3(h3), -1, 1) in fp16; L4 matmuls follow per k
                logits = plog.tile([16, BS], F32, tag="logits")
                ps4 = [ppsum.tile([128, 512], F32, tag="ps", name=f"ps4_{n}")
                       for n in range(NB)]
                y3dbg = None
                bd3 = _bounds(3)
                for ci in range(len(bd3) - 1):
                    emit_stats_pre(3, ci, True, False)
                for ci in range(len(bd3) - 1):
                    emit_stats_post(3, ci, False)
                    s = stats[(3, ci)]
                    for k in range(bd3[ci], bd3[ci + 1]):
                        j = k - bd3[ci]
                        scr = pscr.tile([128, BS], F32, tag="scr", name=f"y3s_{k}")
                        nc.scalar.activation(scr[:], h3[:, k * BS:(k + 1) * BS],
                                             ACT.Identity, bias=s["c"][:, j:j + 1],
                                             scale=s["rp"][:, j:j + 1])
                        y3k = py3.tile([128, BS], F16, tag="y3", name=f"y3_{k}")
                        nc.vector.tensor_scalar(out=y3k[:], in0=scr[:],
                                                scalar1=-1.0, scalar2=1.0,
                                                op0=ALU.max, op1=ALU.min)
                        if k == 0:
                            y3dbg = y3k
                        if stage >= 5:
                            for n in range(NB):
                                nc.tensor.matmul(
                                    ps4[n][:C, :], w4f[:, k * C:(k + 1) * C],
                                    y3k[:, n * 512:(n + 1) * 512],
                                    start=(k == 0), stop=(k == KH - 1))
                if stage == 4:
                    debug_out(y3dbg[:C, :], cast=True)

            if stage >= 5:
                # ===== logits + log-softmax, 4 chunks of 256 cols =====
                for qq in range(4):
                    bank = ps4[qq // 2]
                    bsl = slice((qq % 2) * 256, (qq % 2) * 256 + 256)
                    qsl = slice(qq * 256, (qq + 1) * 256)
                    # logits on DVE (PSUM read) in parallel with exp on ACT
                    nc.vector.tensor_scalar(out=logits[:C, qsl], in0=bank[:C, bsl],
                                            scalar1=b4s[:C, :], scalar2=None,
                                            op0=ALU.add)
                    e_q = ptail.tile([16, 256], F32, tag="tl", name=f"e_{qq}")
                    nc.scalar.activation(e_q[:C, :], bank[:C, bsl], ACT.Exp,
                                         bias=b4s[:C, :], scale=1.0)
                    ps5 = ppsum.tile([128, 256], F32, tag="ps", name=f"ps5_{qq}")
                    nc.tensor.matmul(ps5[:1, :], ones10[:C, :], e_q[:C, :],
                                     start=True, stop=True)
                    lse_q = ptail.tile([16, 256], F32, tag="tl", name=f"lse_{qq}")
                    nc.scalar.activation(lse_q[:1, :], ps5[:1, :], ACT.Ln)
                    ps6 = ppsum.tile([128, 256], F32, tag="ps", name=f"ps6_{qq}")
                    nc.tensor.matmul(ps6[:C, :], onesC[:1, :C], lse_q[:1, :],
                                     start=True, stop=True)
                    outs_q = ptail.tile([16, 256], F32, tag="tl", name=f"o_{qq}")
                    nc.vector.tensor_tensor(outs_q[:C, :], logits[:C, qsl],
                                            ps6[:C, :], op=ALU.subtract)
                    if stage >= 6:
                        nc.sync.dma_start(out_d[:, qsl], outs_q[:C, :])
                if stage == 5:
                    debug_out(logits[:C, :])

    nc.compile()
    return nc


def _prep_inputs(x, W1, b1, g1, bt1, W2, b2, g2, bt2, W3, b3, g3, bt3, W4, b4):
    """Host-side sharding + layout prep (sign, fp8 cast, p-major packing)."""
    def as32(a):
        return np.ascontiguousarray(np.asarray(a, dtype=np.float32))

    f8 = ml_dtypes.float8_e4m3

    def sgn(w):
        return np.where(np.asarray(w) >= 0, np.float32(1.0), np.float32(-1.0))

    def pack_w(w, kt, uks=()):
        # [H_out, K] -> [128, (H_out/128) * K] with per-m-tile p-major blocks.
        # uks: k-blocks whose activations come {0,1}-encoded -> weights +-2.
        s = sgn(w).reshape(-1, 128, kt, 128)            # [m, c, k, p]
        if uks:
            s[:, :, sorted(uks), :] *= 2.0
        s = s.transpose(0, 3, 2, 1).reshape(s.shape[0], 128, kt * 128)
        return np.ascontiguousarray(
            s.transpose(1, 0, 2).reshape(128, -1)).astype(f8)

    def ok(g, bt):
        g, bt = np.asarray(g), np.asarray(bt)
        return bool(not np.any(bt) and np.all(g > 0))

    def ucorr(w, uks):
        # bias correction: -sum over u-encoded k-blocks of sign(w)
        if not uks:
            return 0.0
        s = sgn(w).reshape(w.shape[0], -1, 128)
        return s[:, sorted(uks), :].sum(axis=(1, 2))

    x = as32(x)
    u2 = _u_ks(1) if ok(g1, bt1) else set()
    u3 = _u_ks(2) if ok(g2, bt2) else set()
    W2, W3, b2, b3 = as32(W2), as32(W3), as32(b2), as32(b3)
    shared = {
        "w1pk": pack_w(as32(W1), KD),
        "w2pk": pack_w(W2, KH, u2),
        "w3pk": pack_w(W3, KH, u3),
    }
    b2 = b2 - ucorr(W2, u2)
    b3 = b3 - ucorr(W3, u3)
    cvecs = (b1, g1, bt1, b2, g2, bt2, b3, g3, bt3)
    cpk = np.empty((128, KH * len(cvecs)), np.float32)
    for i, v in enumerate(cvecs):
        cpk[:, i * KH:(i + 1) * KH] = as32(v).reshape(KH, 128).T
    shared["cpk"] = cpk
    w4T = np.ascontiguousarray(as32(W4).T)          # [H, C]
    w4pk = np.empty((128, C * KH), np.float16)
    for k in range(KH):
        w4pk[:, k * C:(k + 1) * C] = w4T[k * 128:(k + 1) * 128, :].astype(np.float16)
    shared["w4pk"] = w4pk
    b4p = np.zeros((16, 1), np.float32)
    b4p[:C, 0] = as32(b4).reshape(-1)
    shared["c_b4"] = b4p

    in_maps = []
    for cr in range(NCORES):
        xT = np.ascontiguousarray(x[cr * BS:(cr + 1) * BS].T)     # [D, BS]
        hi = xT.astype(np.float16)
        lo8 = ((xT - hi.astype(np.float32)) * 4096.0).astype(f8)
        # p-major pack: [D, BS] -> [128, KD*BS]
        hi_pk = np.ascontiguousarray(
            hi.reshape(KD, 128, BS).transpose(1, 0, 2).reshape(128, KD * BS))
        lo_pk = np.ascontiguousarray(
            lo8.reshape(KD, 128, BS).transpose(1, 0, 2).reshape(128, KD * BS))
        m = dict(shared)
        m["xT_hi"] = hi_pk
        m["xT_lo8"] = lo_pk
        in_maps.append(m)
    return in_maps


def _fast_flags(inputs):
    """Mean-only BN boundary valid when beta==0 and gamma>0."""
    def ok(g, bt):
        g, bt = np.asarray(g), np.asarray(bt)
        return bool(not np.any(bt) and np.all(g > 0))

    return (ok(inputs["g1"], inputs["bt1"]), ok(inputs["g2"], inputs["bt2"]))


def kernel(**inputs) -> np.ndarray:
    from concourse.bass_utils import run_bass_kernel_spmd

    fast = _fast_flags(inputs)
    if _CACHE.get("fast") != fast:
        _CACHE["nc"] = _build(fast=fast)
        _CACHE["fast"] = fast
    nc = _CACHE["nc"]
    in_maps = _prep_inputs(**inputs)
    res = run_bass_kernel_spmd(nc, in_maps, list(range(NCORES)))
    out = np.concatenate([res.results[c]["outT"].T for c in range(NCORES)], axis=0)
    return out.astype(np.float32)


# revision 17
# speedup vs baseline: 1.0251x; 1.0148x over previous
"""Trainium2 Bass kernel for nn_BinarizedCifar10MLP — v3.

Data-parallel over batch (8192/8 = 1024 rows/core), feature-major layout.

vs v2 (573us):
  - All weight signing moved to the HOST: W1 ships as fp8e4 +-1 (6.3MB,
    was 12.6MB bf16 + on-device sign), W2/W3 ship as fp8e4 +-1 in DR
    layout (no bf16 read + sign + DRAM round-trip prepass at all).
  - DMA queue discipline: bulk loads (x, W1, Wdr) ride the ACT hwdge
    queue; the sync queue carries only AR traffic + consts + final out.
    W1 m0 is FIRST on the queue (v2 had it behind all 9.4MB of x -> 43us
    PE stall at start); x chunks interleave with W1 m0 sub-tiles and the
    L1 m-loop consumes x chunk-by-chunk, so the PE rides the DMA wave.
  - 3-chunk BN-stat AllReduce for L2/L3 (m 0-9 / 10-13 / 14-15): AR-A
    fires at ~60% of the (short) layer instead of 87%, landing before
    the layer ends; quad-grouped phase-major matmul emission gives the
    PE a 4-m-tile runway on already-signed k-tiles while the tail AR
    lands. Stat sums are n-pair-reduced BEFORE the AR (half payload).
  - log-softmax tail in 4 chunks of 256 cols, exp computed straight
    from PSUM in parallel with the logits drain (DVE reads PSUM).
"""

import sys

sys.path.insert(0, "/opt/trn_rl_repo")

import numpy as np
import ml_dtypes

B, D, H, C = 8192, 3 * 32 * 32, 2048, 10
EPS = 1e-5
NCORES = 8
BS = B // NCORES          # 1024 batch rows per core
KD = D // 128             # 24 k-tiles over input dim
KH = H // 128             # 16 k-tiles over hidden dim
NB = BS // 512            # 2 free-dim chunks of 512
CHK = {1: (14, 16), 2: (10, 14, 16), 3: (10, 14, 16)}  # AR chunk end bounds
XCH = 4                   # x DMA chunks (6 k-tiles each)
KC = KD // XCH            # 6 k-tiles per x chunk

_CACHE = {}


def _bounds(l):
    return (0,) + CHK[l]


def _u_ks(l_prod):
    """k-tiles of the layer-l_prod sign output that are {0,1}-encoded (single
    DVE is_ge op); the consumer weights are host-scaled to +-2 on those blocks
    with the -rowsum(sign W) correction folded into the bias."""
    bd = _bounds(l_prod)
    nch = len(bd) - 1
    dve = {nch - 1} if l_prod == 1 else set(range(1, nch))
    ks = set()
    for ci in range(nch):
        for k in range(bd[ci], bd[ci + 1]):
            if ci in dve or k % 2 == 0:
                ks.add(k)
    return ks


def _build(stage=7, fast=(False, False)):
    import concourse.bacc as bacc
    import concourse.mybir as mybir
    import concourse.tile as tile

    F32 = mybir.dt.float32
    F16 = mybir.dt.float16
    F8E4 = mybir.dt.float8e4
    F8E5 = mybir.dt.float8e5
    DRM = mybir.MatmulPerfMode.DoubleRow
    ACT = mybir.ActivationFunctionType
    ALU = mybir.AluOpType
    RG = [list(range(NCORES))]

    nc = bacc.Bacc("TRN2", target_bir_lowering=False, debug=False, num_devices=NCORES)

    # ---- I/O ----
    xhi_d = nc.dram_tensor("xT_hi", [128, KD * BS], F16, kind="ExternalInput").ap()
    xlo_d = nc.dram_tensor("xT_lo8", [128, KD * BS], F8E4, kind="ExternalInput").ap()
    w1_d = nc.dram_tensor("w1pk", [128, KH * KD * 128], F8E4, kind="ExternalInput").ap()
    w2_d = nc.dram_tensor("w2pk", [128, KH * KH * 128], F8E4, kind="ExternalInput").ap()
    w3_d = nc.dram_tensor("w3pk", [128, KH * KH * 128], F8E4, kind="ExternalInput").ap()
    CNAMES = ("b1", "g1", "bt1", "b2", "g2", "bt2", "b3", "g3", "bt3")
    cpk_d = nc.dram_tensor("cpk", [128, KH * len(CNAMES)], F32, kind="ExternalInput").ap()
    w4pk_d = nc.dram_tensor("w4pk", [128, C * KH], F16, kind="ExternalInput").ap()
    b4_d = nc.dram_tensor("c_b4", [16, 1], F32, kind="ExternalInput").ap()
    out_d = nc.dram_tensor("outT", [C, BS], F32, kind="ExternalOutput").ap()

    wl_d = {2: w2_d, 3: w3_d}

    with tile.TileContext(nc) as tc:
        with (
            tc.tile_pool(name="pconst", bufs=1) as pconst,
            tc.tile_pool(name="pstat", bufs=1) as pstat,
            tc.tile_pool(name="plog", bufs=1) as plog,
            tc.tile_pool(name="ptail", bufs=6) as ptail,
            tc.tile_pool(name="pscr", bufs=3) as pscr,
            tc.tile_pool(name="pw", bufs=4) as pw,
            tc.tile_pool(name="pw8", bufs=2) as pw8,
            tc.tile_pool(name="pwdr", bufs=8) as pwdr,
            tc.tile_pool(name="py3", bufs=4) as py3,
            tc.tile_pool(name="ph", bufs=1) as ph,
            tc.tile_pool(name="pa", bufs=1) as pa,
            tc.tile_pool(name="pb", bufs=1) as pb,
            tc.tile_pool(name="ppsum", bufs=8, space="PSUM") as ppsum,
            tc.tile_pool(name="pdram", bufs=16, space="DRAM") as pdram,
        ):
            # ---- warmup AllReduce: absorbs ncfw first-collective staging ----
            wuin = pdram.tile([128, 4], F32, tag="wuin")
            wuout = pdram.tile([128, 4], F32, tag="wuout")
            wusrc = pstat.tile([128, 4], F32, tag="wusrc")
            nc.vector.memset(wusrc[:], 0.0)
            nc.sync.dma_start(wuin[:], wusrc[:])
            nc.gpsimd.collective_compute(
                "AllReduce", ALU.add, replica_groups=RG,
                ins=[wuin.opt()], outs=[wuout.opt()])

            # ---- constants (sync queue; small, land early) ----
            cpk = pconst.tile([128, KH * len(CNAMES)], F32, tag="cpk")
            nc.sync.dma_start(cpk[:], cpk_d)
            cons = {name: cpk[:, i * KH:(i + 1) * KH] for i, name in enumerate(CNAMES)}
            b4s = pconst.tile([16, 1], F32, tag="b4")
            nc.sync.dma_start(b4s[:], b4_d)
            ones10 = pconst.tile([16, 1], F32, tag="ones10")
            nc.vector.memset(ones10[:], 1.0)
            onesC = pconst.tile([1, 16], F32, tag="onesC")
            nc.vector.memset(onesC[:], 1.0)

            # ---- bulk loads: scalar (ACT hwdge) queue ----
            # W1 m0 interleaved chunk-wise with x so the PE starts ~5us in.
            xhi = pa.tile([128, KD * BS], F16, tag="pa", name="xhi")
            xlo8 = pb.tile([128, KD * BS], F8E4, tag="pb", name="xlo8")
            w1_pf = {}
            for m in range(3):
                w1_pf[m] = pw.tile([128, KD * 128], F8E4, tag="w", name=f"w1_{m}")
            for c in range(XCH):
                nc.scalar.dma_start(
                    w1_pf[0][:, c * KC * 128:(c + 1) * KC * 128],
                    w1_d[:, c * KC * 128:(c + 1) * KC * 128])
                sl = slice(c * KC * BS, (c + 1) * KC * BS)
                nc.scalar.dma_start(xhi[:, sl], xhi_d[:, sl])
                nc.scalar.dma_start(xlo8[:, sl], xlo_d[:, sl])
            for m in range(1, 3):
                nc.scalar.dma_start(
                    w1_pf[m][:], w1_d[:, m * KD * 128:(m + 1) * KD * 128])
            xlo8v = xlo8[:].rearrange("p (k c) -> p k c", c=BS)

            # Wdr stream: fp8 +-1 DR-layout weights for L2/L3, 8-deep ring.
            # gens 0..15 = L2 m0..15, 16..31 = L3 m0..15.
            wdr_pf = {}

            def emit_wdr(gen):
                l, m = (2, gen) if gen < 16 else (3, gen - 16)
                w8t = pwdr.tile([128, KH * 128], F8E4, tag="wdr", name=f"wdr_{l}_{m}")
                nc.scalar.dma_start(w8t[:], wl_d[l][:, m * 2048:(m + 1) * 2048])
                wdr_pf[(l, m)] = w8t

            for gen in range(3):
                emit_wdr(gen)

            w4f = pconst.tile([128, C * KH], F16, tag="w4f")
            nc.scalar.dma_start(w4f[:], w4pk_d)

            parts = {}
            gchunk = {}     # (l, ci) -> allreduced pre-reduced stats tile
            stats = {}      # (l, ci) -> dict of stat tiles
            arouts = {}

            def emit_ar_fire(l, ci, do_sq):
                """n-pair-reduce sums, append sq cols -> DRAM -> AllReduce."""
                bd = _bounds(l)
                c0, c1 = bd[ci], bd[ci + 1]
                nm = c1 - c0
                w = 2 * nm if do_sq else nm
                red = pstat.tile([128, nm], F32, tag=f"red{l}{ci}", name=f"red{l}{ci}")
                nc.vector.tensor_reduce(
                    red[:],
                    parts[l][:, 2 * c0:2 * c1].rearrange("p (m n) -> p m n", n=2),
                    axis=mybir.AxisListType.X, op=ALU.add)
                arin = pdram.tile([128, w], F32, tag=f"arin{l}{ci}")
                arout = pdram.tile([128, w], F32, tag=f"arout{l}{ci}")
                nc.sync.dma_start(arin[:, 0:nm], red[:])
                if do_sq:
                    nc.sync.dma_start(arin[:, nm:w], parts[l][:, 32 + c0:32 + c1])
                nc.gpsimd.collective_compute(
                    "AllReduce", ALU.add, replica_groups=RG,
                    ins=[arin.opt()], outs=[arout.opt()])
                arouts[(l, ci)] = (arout, w)

            def emit_ar_land(l, ci):
                arout, w = arouts[(l, ci)]
                g_t = pstat.tile([128, w], F32, tag=f"g{l}{ci}", name=f"g{l}{ci}")
                nc.sync.dma_start(g_t[:], arout[:])
                gchunk[(l, ci)] = g_t

            def _st(l, ci, tag, nm):
                return pstat.tile([128, nm], F32, tag=f"{tag}{l}{ci}",
                                  name=f"{tag}{l}{ci}")

            def emit_stats_pre(l, ci, do_sq, fastl):
                """DVE-only stats from the pre-reduced AR result (safe to emit
                mid-loop: no ACT ops to block later drains)."""
                g_t = gchunk[(l, ci)]
                bd = _bounds(l)
                nm = bd[ci + 1] - bd[ci]
                m1 = _st(l, ci, "m1", nm)
                nc.vector.tensor_scalar_mul(m1[:], g_t[:, 0:nm], 1.0 / B)
                if fastl and not do_sq:
                    negm = _st(l, ci, "negm", nm)
                    nc.vector.tensor_scalar_mul(negm[:], g_t[:, 0:nm], -1.0 / B)
                    stats[(l, ci)] = dict(m1=m1, negm=negm, fast=True)
                    return
                msq, m1sq, v = (_st(l, ci, x, nm) for x in ("msq", "m1sq", "v"))
                nc.vector.tensor_scalar_mul(msq[:], g_t[:, nm:2 * nm], 1.0 / B)
                nc.vector.tensor_tensor(m1sq[:], m1[:], m1[:], op=ALU.mult)
                nc.vector.tensor_tensor(v[:], msq[:], m1sq[:], op=ALU.subtract)
                nc.vector.tensor_scalar_add(v[:], v[:], EPS)
                stats[(l, ci)] = dict(m1=m1, v=v, fast=False)

            def emit_stats_post(l, ci, fastl):
                """ACT sqrt + downstream scale/bias (emit after the layer's
                drains so the ACT queue never blocks on a pending AR)."""
                d = stats[(l, ci)]
                if d["fast"]:
                    return
                bd = _bounds(l)
                c0 = bd[ci]
                nm = bd[ci + 1] - c0
                gcol = cons[f"g{l}"][:, c0:c0 + nm]
                btcol = cons[f"bt{l}"][:, c0:c0 + nm]
                m1, v = d["m1"], d["v"]
                r, rp, mt, cc = (_st(l, ci, x, nm) for x in ("r", "rp", "mt", "c"))
                sq = _st(l, ci, "sq", nm)
                nc.scalar.activation(sq[:], v[:], ACT.Sqrt)
                nc.vector.reciprocal(r[:], sq[:])
                nc.vector.tensor_tensor(rp[:], gcol, r[:], op=ALU.mult)
                nc.vector.tensor_tensor(mt[:], m1[:], rp[:], op=ALU.mult)
                nc.vector.tensor_tensor(cc[:], btcol, mt[:], op=ALU.subtract)
                d.update(rp=rp, c=cc)
                if l < 3:
                    gi, u, u2, tthr, s, s2, sneg = (
                        _st(l, ci, x, nm)
                        for x in ("gi", "u", "u2", "tthr", "s", "s2", "sneg"))
                    nc.vector.reciprocal(gi[:], gcol)
                    nc.vector.tensor_tensor(u[:], btcol, gi[:], op=ALU.mult)
                    nc.vector.tensor_tensor(u2[:], u[:], sq[:], op=ALU.mult)
                    nc.vector.tensor_tensor(tthr[:], m1[:], u2[:], op=ALU.subtract)
                    nc.scalar.activation(s[:], gcol, ACT.Sign)
                    nc.vector.tensor_scalar_mul(s2[:], s[:], 2.0)
                    nc.vector.tensor_scalar_mul(sneg[:], s[:], -1.0)
                    d.update(tthr=tthr, s2=s2, sneg=sneg)

            def chunk_of(l, k):
                bd = _bounds(l)
                for ci in range(len(bd) - 1):
                    if k < bd[ci + 1]:
                        return ci, k - bd[ci]

            def sign_wave(l, dst3, h_t, krange, dve_only=False):
                """a[:, k, :] = sign-of-bn for k in krange; alternate ACT/DVE.
                dve_only for AR-end-gated chunks: keeps the ACT queue free of
                AR-gated ops so the next layer's drains + wdr DMA triggers
                (which ride the ACT queue in-order) are never blocked."""
                for k in krange:
                    ci, j = chunk_of(l, k)
                    s = stats[(l, ci)]
                    hsl = h_t[:, k * BS:(k + 1) * BS]
                    dst = dst3[:, k, :]
                    if k % 2 == 1 and not dve_only:
                        scale = 1.0 if s["fast"] else s["rp"][:, j:j + 1]
                        bias = s["negm"][:, j:j + 1] if s["fast"] else s["c"][:, j:j + 1]
                        nc.scalar.activation(dst, hsl, ACT.Sign, bias=bias, scale=scale)
                    elif s["fast"]:
                        # {0,1} encoding: weights are +-2 with bias correction
                        nc.vector.tensor_scalar(out=dst, in0=hsl,
                                                scalar1=s["m1"][:, j:j + 1],
                                                scalar2=None, op0=ALU.is_ge)
                    else:
                        thr = s["tthr"][:, j:j + 1]
                        bt_ = pscr.tile([128, BS], F16, tag="scr", name=f"sgb_{l}_{k}")
                        nc.vector.tensor_scalar(out=bt_[:], in0=hsl, scalar1=thr,
                                                scalar2=None, op0=ALU.is_ge)
                        nc.vector.tensor_scalar(out=dst, in0=bt_[:],
                                                scalar1=s["s2"][:, j:j + 1],
                                                scalar2=s["sneg"][:, j:j + 1],
                                                op0=ALU.mult, op1=ALU.add)

            def drain(l, m, n, ps, h_t, do_sq):
                hs = h_t[:, m * BS + n * 512: m * BS + n * 512 + 512]
                col = 2 * m + n
                nc.scalar.activation(hs, ps[:], ACT.Identity,
                                     bias=cons[f"b{l}"][:, m:m + 1], scale=1.0,
                                     accum_out=parts[l][:, col:col + 1])

            def msum(l, m, h_t, do_sq):
                """L3 sumsq: one fused DVE op (the ACT-Square+accum pair was
                2.8us/m-tile and made L3 ACT-bound)."""
                if not do_sq:
                    return
                hrow = h_t[:, m * BS:(m + 1) * BS]
                scr = pscr.tile([128, BS], F32, tag="scr", name=f"sq_{l}_{m}")
                nc.vector.tensor_tensor(scr[:], hrow, hrow, op=ALU.mult)
                nc.vector.tensor_reduce(
                    parts[l][:, 32 + m:32 + m + 1],
                    scr[:].rearrange("p (a b) -> p a b", a=1),
                    axis=mybir.AxisListType.X, op=ALU.add)

            def debug_out(src_ap, cast=False):
                if cast:
                    t = pscr.tile([128, BS], F32, tag="scr", name="dbgcast")
                    nc.vector.tensor_copy(t[:C, :], src_ap)
                    src_ap = t[:C, :]
                nc.sync.dma_start(out_d[:], src_ap)

            # ===================== Layer 1 =====================
            h1 = ph.tile([128, KH * BS], F32, tag="ph", name="h1")
            parts[1] = pstat.tile([128, 64], F32, tag="parts1", name="parts1")
            do_sq1 = not fast[0]
            bd1 = _bounds(1)

            def l1_mtile_alloc(m):
                st = {}
                st["wst"] = w1_pf.pop(m)
                st["w8lo"] = pw8.tile([128, KD * 128], F8E5, tag="w8",
                                      name=f"w8lo_{m}")
                st["w8lov"] = st["w8lo"][:].rearrange("p (k c) -> p k c", c=128)
                st["pss"] = [ppsum.tile([128, 512], F32, tag="ps",
                                        name=f"ps1_{m}_{n}") for n in range(NB)]
                return st

            def l1_chunk(st, c):
                wst, w8lo, w8lov, pss = (st["wst"], st["w8lo"], st["w8lov"],
                                         st["pss"])
                for k in range(c * KC, (c + 1) * KC):
                    lhsT = wst[:, k * 128:(k + 1) * 128]
                    for n in range(NB):
                        nc.tensor.matmul(
                            pss[n][:], lhsT,
                            xhi[:, k * BS + n * 512: k * BS + n * 512 + 512],
                            start=(k == 0), stop=False)
                nc.vector.tensor_scalar_mul(
                    w8lo[:, c * KC * 128:(c + 1) * KC * 128],
                    wst[:, c * KC * 128:(c + 1) * KC * 128], 2.0 ** -12)
                for t in range(c * KC // 2, (c + 1) * KC // 2):
                    lhsT = w8lov[:, 2 * t:2 * t + 2, :]
                    for n in range(NB):
                        nc.tensor.matmul(
                            pss[n][:], lhsT,
                            xlo8v[:, 2 * t:2 * t + 2, n * 512:n * 512 + 512],
                            start=False, stop=(t == KD // 2 - 1), perf_mode=DRM)

            def l1_tail(m, st):
                # W1 prefetch 3 ahead; Wdr gens 3..7 during m=8..12
                if m + 3 < KH and m + 3 not in w1_pf:
                    w1_pf[m + 3] = pw.tile([128, KD * 128], F8E4, tag="w",
                                           name=f"w1_{m + 3}")
                    nc.scalar.dma_start(
                        w1_pf[m + 3][:],
                        w1_d[:, (m + 3) * KD * 128:(m + 4) * KD * 128])
                if 8 <= m <= 12:
                    emit_wdr(m - 5)
                for n in range(NB):
                    drain(1, m, n, st["pss"][n], h1, do_sq1)
                msum(1, m, h1, do_sq1)
                for ci in range(len(bd1) - 1):
                    if m == bd1[ci + 1] - 1:
                        emit_ar_fire(1, ci, do_sq1)
                        if ci > 0:
                            emit_ar_land(1, ci - 1)

            # m=0,1 interleaved per x chunk: the PE rides the incoming DMA wave
            st01 = {m: l1_mtile_alloc(m) for m in (0, 1)}
            for c in range(XCH):
                for m in (0, 1):
                    l1_chunk(st01[m], c)
            for m in (0, 1):
                l1_tail(m, st01[m])
            for m in range(2, KH):
                st = l1_mtile_alloc(m)
                for c in range(XCH):
                    l1_chunk(st, c)
                l1_tail(m, st)
            emit_ar_land(1, len(bd1) - 2)

            if stage == 1:
                debug_out(h1[:C, :BS])

            a2 = pa.tile([128, KH, BS], F8E4, tag="pa", name="a2")
            nch1 = len(bd1) - 1
            for ci in range(nch1):
                emit_stats_pre(1, ci, do_sq1, fast[0])
                emit_stats_post(1, ci, fast[0])
                sign_wave(1, a2, h1, range(bd1[ci], bd1[ci + 1]),
                          dve_only=(ci == nch1 - 1))
            if stage == 2:
                debug_out(a2[:C, 0, :], cast=True)

            # ===================== Layers 2, 3 =====================
            def dense_dr(l, a_in):
                h_t = ph.tile([128, KH * BS], F32, tag="ph", name=f"h{l}")
                parts[l] = pstat.tile([128, 64], F32, tag=f"parts{l}", name=f"parts{l}")
                do_sq = (l == 3) or not fast[l - 1]
                bd = _bounds(l)
                # t-phases matching the PRODUCING layer's sign chunks
                pb_in = _bounds(l - 1)
                tph = [(pb_in[i] // 2, pb_in[i + 1] // 2) for i in range(len(pb_in) - 1)]
                for q in range(KH // 4):
                    ms = range(4 * q, 4 * q + 4)
                    pss = {m: [ppsum.tile([128, 512], F32, tag="ps",
                                          name=f"ps{l}_{m}_{n}") for n in range(NB)]
                           for m in ms}
                    w8 = {m: wdr_pf.pop((l, m)) for m in ms}
                    for ta, tb in tph:
                        for m in ms:
                            w8v = w8[m][:].rearrange("p (k c) -> p k c", c=128)
                            for t in range(ta, tb):
                                lhsT = w8v[:, 2 * t:2 * t + 2, :]
                                for n in range(NB):
                                    nc.tensor.matmul(
                                        pss[m][n][:], lhsT,
                                        a_in[:, 2 * t:2 * t + 2, n * 512:n * 512 + 512],
                                        start=(t == 0), stop=(t == KH // 2 - 1),
                                        perf_mode=DRM)
                    for m in ms:
                        gen = (l - 2) * 16 + m + 8
                        if gen < 32:
                            emit_wdr(gen)
                        for n in range(NB):
                            drain(l, m, n, pss[m][n], h_t, do_sq)
                        msum(l, m, h_t, do_sq)
                        for ci in range(len(bd) - 1):
                            if m == bd[ci + 1] - 1:
                                emit_ar_fire(l, ci, do_sq)
                                if ci > 0:
                                    emit_ar_land(l, ci - 1)
                emit_ar_land(l, len(bd) - 2)
                return h_t

            if stage >= 3:
                h2 = dense_dr(2, a2[:])
                a3 = pb.tile([128, KH, BS], F8E4, tag="pb", name="a3")
                bd2 = _bounds(2)
                for ci in range(len(bd2) - 1):
                    emit_stats_pre(2, ci, not fast[1], fast[1])
                    emit_stats_post(2, ci, fast[1])
                    sign_wave(2, a3, h2, range(bd2[ci], bd2[ci + 1]),
                              dve_only=(ci >= 1))
                if stage == 3:
                    debug_out(a3[:C, 0, :], cast=True)

            if stage >= 4:
                h3 = dense_dr(3, a3[:])
                # y3 = clip(bn3(h3), -1, 1) in fp16; L4 matmuls follow per k
                logits = plog.tile([16, BS], F32, tag="logits")
                ps4 = [ppsum.tile([128, 512], F32, tag="ps", name=f"ps4_{n}")
                       for n in range(NB)]
                y3dbg = None
                bd3 = _bounds(3)
                for ci in range(len(bd3) - 1):
                    emit_stats_pre(3, ci, True, False)
                for ci in range(len(bd3) - 1):
                    emit_stats_post(3, ci, False)
                    s = stats[(3, ci)]
                    for k in range(bd3[ci], bd3[ci + 1]):
                        j = k - bd3[ci]
                        scr = pscr.tile([128, BS], F32, tag="scr", name=f"y3s_{k}")
                        if k % 4 == 3:
                            # full-DVE path: offloads the serial ACT chain
                            nc.vector.tensor_scalar(
                                out=scr[:], in0=h3[:, k * BS:(k + 1) * BS],
                                scalar1=s["rp"][:, j:j + 1],
                                scalar2=s["c"][:, j:j + 1],
                                op0=ALU.mult, op1=ALU.add)
                        else:
                            nc.scalar.activation(scr[:],
                                                 h3[:, k * BS:(k + 1) * BS],
                                                 ACT.Identity,
                                                 bias=s["c"][:, j:j + 1],
                                                 scale=s["rp"][:, j:j + 1])
                        y3k = py3.tile([128, BS], F16, tag="y3", name=f"y3_{k}")
                        nc.vector.tensor_scalar(out=y3k[:], in0=scr[:],
                                                scalar1=-1.0, scalar2=1.0,
                                                op0=ALU.max, op1=ALU.min)
                        if k == 0:
                            y3dbg = y3k
                        if stage >= 5:
                            for n in range(NB):
                                nc.tensor.matmul(
                                    ps4[n][:C, :], w4f[:, k * C:(k + 1) * C],
                                    y3k[:, n * 512:(n + 1) * 512],
                                    start=(k == 0), stop=(k == KH - 1))
                if stage == 4:
                    debug_out(y3dbg[:C, :], cast=True)

            if stage >= 5:
                # ===== logits + log-softmax, 4 chunks of 256 cols =====
                for qq in range(4):
                    bank = ps4[qq // 2]
                    bsl = slice((qq % 2) * 256, (qq % 2) * 256 + 256)
                    qsl = slice(qq * 256, (qq + 1) * 256)
                    # logits on DVE (PSUM read) in parallel with exp on ACT
                    nc.vector.tensor_scalar(out=logits[:C, qsl], in0=bank[:C, bsl],
                                            scalar1=b4s[:C, :], scalar2=None,
                                            op0=ALU.add)
                    e_q = ptail.tile([16, 256], F32, tag="tl", name=f"e_{qq}")
                    nc.scalar.activation(e_q[:C, :], bank[:C, bsl], ACT.Exp,
                                         bias=b4s[:C, :], scale=1.0)
                    ps5 = ppsum.tile([128, 256], F32, tag="ps", name=f"ps5_{qq}")
                    nc.tensor.matmul(ps5[:1, :], ones10[:C, :], e_q[:C, :],
                                     start=True, stop=True)
                    lse_q = ptail.tile([16, 256], F32, tag="tl", name=f"lse_{qq}")
                    nc.scalar.activation(lse_q[:1, :], ps5[:1, :], ACT.Ln)
                    ps6 = ppsum.tile([128, 256], F32, tag="ps", name=f"ps6_{qq}")
                    nc.tensor.matmul(ps6[:C, :], onesC[:1, :C], lse_q[:1, :],
                                     start=True, stop=True)
                    outs_q = ptail.tile([16, 256], F32, tag="tl", name=f"o_{qq}")
                    nc.vector.tensor_tensor(outs_q[:C, :], logits[:C, qsl],
                                            ps6[:C, :], op=ALU.subtract)
                    if stage >= 6:
                        nc.sync.dma_start(out_d[:, qsl], outs_q[:C, :])
                if stage == 5:
                    debug_out(logits[:C, :])

    nc.compile()
    return nc


def _prep_inputs(x, W1, b1, g1, bt1, W2, b2, g2, bt2, W3, b3, g3, bt3, W4, b4):
    """Host-side sharding + layout prep (sign, fp8 cast, p-major packing)."""
    def as32(a):
        return np.ascontiguousarray(np.asarray(a, dtype=np.float32))

    f8 = ml_dtypes.float8_e4m3

    def sgn(w):
        return np.where(np.asarray(w) >= 0, np.float32(1.0), np.float32(-1.0))

    def pack_w(w, kt, uks=()):
        # [H_out, K] -> [128, (H_out/128) * K] with per-m-tile p-major blocks.
        # uks: k-blocks whose activations come {0,1}-encoded -> weights +-2.
        s = sgn(w).reshape(-1, 128, kt, 128)            # [m, c, k, p]
        if uks:
            s[:, :, sorted(uks), :] *= 2.0
        s = s.transpose(0, 3, 2, 1).reshape(s.shape[0], 128, kt * 128)
        return np.ascontiguousarray(
            s.transpose(1, 0, 2).reshape(128, -1)).astype(f8)

    def ok(g, bt):
        g, bt = np.asarray(g), np.asarray(bt)
        return bool(not np.any(bt) and np.all(g > 0))

    def ucorr(w, uks):
        # bias correction: -sum over u-encoded k-blocks of sign(w)
        if not uks:
            return 0.0
        s = sgn(w).reshape(w.shape[0], -1, 128)
        return s[:, sorted(uks), :].sum(axis=(1, 2))

    x = as32(x)
    u2 = _u_ks(1) if ok(g1, bt1) else set()
    u3 = _u_ks(2) if ok(g2, bt2) else set()
    W2, W3, b2, b3 = as32(W2), as32(W3), as32(b2), as32(b3)
    shared = {
        "w1pk": pack_w(as32(W1), KD),
        "w2pk": pack_w(W2, KH, u2),
        "w3pk": pack_w(W3, KH, u3),
    }
    b2 = b2 - ucorr(W2, u2)
    b3 = b3 - ucorr(W3, u3)
    cvecs = (b1, g1, bt1, b2, g2, bt2, b3, g3, bt3)
    cpk = np.empty((128, KH * len(cvecs)), np.float32)
    for i, v in enumerate(cvecs):
        cpk[:, i * KH:(i + 1) * KH] = as32(v).reshape(KH, 128).T
    shared["cpk"] = cpk
    w4T = np.ascontiguousarray(as32(W4).T)          # [H, C]
    w4pk = np.empty((128, C * KH), np.float16)
    for k in range(KH):
        w4pk[:, k * C:(k + 1) * C] = w4T[k * 128:(k + 1) * 128, :].astype(np.float16)
    shared["w4pk"] = w4pk
    b4p = np.zeros((16, 1), np.float32)
    b4p[:C, 0] = as32(b4).reshape(-1)
    shared["c_b4"] = b4p

    in_maps = []
    for cr in range(NCORES):
        xT = np.ascontiguousarray(x[cr * BS:(cr + 1) * BS].T)     # [D, BS]
        hi = xT.astype(np.float16)
        lo8 = ((xT - hi.astype(np.float32)) * 4096.0).astype(f8)
        # p-major pack: [D, BS] -> [128, KD*BS]
        hi_pk = np.ascontiguousarray(
            hi.reshape(KD, 128, BS).transpose(1, 0, 2).reshape(128, KD * BS))
        lo_pk = np.ascontiguousarray(
            lo8.reshape(KD, 128, BS).transpose(1, 0, 2).reshape(128, KD * BS))
        m = dict(shared)
        m["xT_hi"] = hi_pk
        m["xT_lo8"] = lo_pk
        in_maps.append(m)
    return in_maps


def _fast_flags(inputs):
    """Mean-only BN boundary valid when beta==0 and gamma>0."""
    def ok(g, bt):
        g, bt = np.asarray(g), np.asarray(bt)
        return bool(not np.any(bt) and np.all(g > 0))

    return (ok(inputs["g1"], inputs["bt1"]), ok(inputs["g2"], inputs["bt2"]))


def kernel(**inputs) -> np.ndarray:
    from concourse.bass_utils import run_bass_kernel_spmd

    fast = _fast_flags(inputs)
    if _CACHE.get("fast") != fast:
        _CACHE["nc"] = _build(fast=fast)
        _CACHE["fast"] = fast
    nc = _CACHE["nc"]
    in_maps = _prep_inputs(**inputs)
    res = run_bass_kernel_spmd(nc, in_maps, list(range(NCORES)))
    out = np.concatenate([res.results[c]["outT"].T for c in range(NCORES)], axis=0)
    return out.astype(np.float32)


# revision 18
# speedup vs baseline: 1.0544x; 1.0286x over previous
"""Trainium2 Bass kernel for nn_BinarizedCifar10MLP — v3.

Data-parallel over batch (8192/8 = 1024 rows/core), feature-major layout.

vs v2 (573us):
  - All weight signing moved to the HOST: W1 ships as fp8e4 +-1 (6.3MB,
    was 12.6MB bf16 + on-device sign), W2/W3 ship as fp8e4 +-1 in DR
    layout (no bf16 read + sign + DRAM round-trip prepass at all).
  - DMA queue discipline: bulk loads (x, W1, Wdr) ride the ACT hwdge
    queue; the sync queue carries only AR traffic + consts + final out.
    W1 m0 is FIRST on the queue (v2 had it behind all 9.4MB of x -> 43us
    PE stall at start); x chunks interleave with W1 m0 sub-tiles and the
    L1 m-loop consumes x chunk-by-chunk, so the PE rides the DMA wave.
  - 3-chunk BN-stat AllReduce for L2/L3 (m 0-9 / 10-13 / 14-15): AR-A
    fires at ~60% of the (short) layer instead of 87%, landing before
    the layer ends; quad-grouped phase-major matmul emission gives the
    PE a 4-m-tile runway on already-signed k-tiles while the tail AR
    lands. Stat sums are n-pair-reduced BEFORE the AR (half payload).
  - log-softmax tail in 4 chunks of 256 cols, exp computed straight
    from PSUM in parallel with the logits drain (DVE reads PSUM).
"""

import sys

sys.path.insert(0, "/opt/trn_rl_repo")

import numpy as np
import ml_dtypes

B, D, H, C = 8192, 3 * 32 * 32, 2048, 10
EPS = 1e-5
NCORES = 8
BS = B // NCORES          # 1024 batch rows per core
KD = D // 128             # 24 k-tiles over input dim
KH = H // 128             # 16 k-tiles over hidden dim
NB = BS // 512            # 2 free-dim chunks of 512
CHK = {1: (14, 16), 2: (10, 16), 3: (10, 16)}  # AR chunk end bounds
XCH = 4                   # x DMA chunks (6 k-tiles each)
KC = KD // XCH            # 6 k-tiles per x chunk

_CACHE = {}


def _bounds(l):
    return (0,) + CHK[l]


def _u_ks(l_prod):
    """k-tiles of the layer-l_prod sign output that are {0,1}-encoded (single
    DVE is_ge op); the consumer weights are host-scaled to +-2 on those blocks
    with the -rowsum(sign W) correction folded into the bias."""
    bd = _bounds(l_prod)
    nch = len(bd) - 1
    dve = {nch - 1} if l_prod == 1 else set(range(1, nch))
    ks = set()
    for ci in range(nch):
        for k in range(bd[ci], bd[ci + 1]):
            if ci in dve or k % 2 == 0:
                ks.add(k)
    return ks


def _build(stage=7, fast=(False, False)):
    import concourse.bacc as bacc
    import concourse.mybir as mybir
    import concourse.tile as tile

    F32 = mybir.dt.float32
    F16 = mybir.dt.float16
    F8E4 = mybir.dt.float8e4
    F8E5 = mybir.dt.float8e5
    DRM = mybir.MatmulPerfMode.DoubleRow
    ACT = mybir.ActivationFunctionType
    ALU = mybir.AluOpType
    RG = [list(range(NCORES))]

    nc = bacc.Bacc("TRN2", target_bir_lowering=False, debug=False, num_devices=NCORES)

    # ---- I/O ----
    xhi_d = nc.dram_tensor("xT_hi", [128, KD * BS], F16, kind="ExternalInput").ap()
    xlo_d = nc.dram_tensor("xT_lo8", [128, KD * BS], F8E4, kind="ExternalInput").ap()
    w1_d = nc.dram_tensor("w1pk", [128, KH * KD * 128], F8E4, kind="ExternalInput").ap()
    w2_d = nc.dram_tensor("w2pk", [128, KH * KH * 128], F8E4, kind="ExternalInput").ap()
    w3_d = nc.dram_tensor("w3pk", [128, KH * KH * 128], F8E4, kind="ExternalInput").ap()
    CNAMES = ("b1", "g1", "bt1", "b2", "g2", "bt2", "b3", "g3", "bt3")
    cpk_d = nc.dram_tensor("cpk", [128, KH * len(CNAMES)], F32, kind="ExternalInput").ap()
    w4pk_d = nc.dram_tensor("w4pk", [128, C * KH], F16, kind="ExternalInput").ap()
    b4_d = nc.dram_tensor("c_b4", [16, 1], F32, kind="ExternalInput").ap()
    out_d = nc.dram_tensor("outT", [C, BS], F32, kind="ExternalOutput").ap()

    wl_d = {2: w2_d, 3: w3_d}

    with tile.TileContext(nc) as tc:
        with (
            tc.tile_pool(name="pconst", bufs=1) as pconst,
            tc.tile_pool(name="pstat", bufs=1) as pstat,
            tc.tile_pool(name="plog", bufs=1) as plog,
            tc.tile_pool(name="ptail", bufs=6) as ptail,
            tc.tile_pool(name="pscr", bufs=3) as pscr,
            tc.tile_pool(name="pw", bufs=4) as pw,
            tc.tile_pool(name="pw8", bufs=2) as pw8,
            tc.tile_pool(name="pwdr", bufs=8) as pwdr,
            tc.tile_pool(name="py3", bufs=4) as py3,
            tc.tile_pool(name="ph", bufs=1) as ph,
            tc.tile_pool(name="pa", bufs=1) as pa,
            tc.tile_pool(name="pb", bufs=1) as pb,
            tc.tile_pool(name="ppsum", bufs=8, space="PSUM") as ppsum,
            tc.tile_pool(name="pdram", bufs=16, space="DRAM") as pdram,
        ):
            # ---- warmup AllReduce: absorbs ncfw first-collective staging ----
            wuin = pdram.tile([128, 4], F32, tag="wuin")
            wuout = pdram.tile([128, 4], F32, tag="wuout")
            wusrc = pstat.tile([128, 4], F32, tag="wusrc")
            nc.vector.memset(wusrc[:], 0.0)
            nc.sync.dma_start(wuin[:], wusrc[:])
            nc.gpsimd.collective_compute(
                "AllReduce", ALU.add, replica_groups=RG,
                ins=[wuin.opt()], outs=[wuout.opt()])

            # ---- constants (sync queue; small, land early) ----
            cpk = pconst.tile([128, KH * len(CNAMES)], F32, tag="cpk")
            nc.sync.dma_start(cpk[:], cpk_d)
            cons = {name: cpk[:, i * KH:(i + 1) * KH] for i, name in enumerate(CNAMES)}
            b4s = pconst.tile([16, 1], F32, tag="b4")
            nc.sync.dma_start(b4s[:], b4_d)
            ones10 = pconst.tile([16, 1], F32, tag="ones10")
            nc.vector.memset(ones10[:], 1.0)
            onesC = pconst.tile([1, 16], F32, tag="onesC")
            nc.vector.memset(onesC[:], 1.0)

            # ---- bulk loads: scalar (ACT hwdge) queue ----
            # W1 m0 interleaved chunk-wise with x so the PE starts ~5us in.
            xhi = pa.tile([128, KD * BS], F16, tag="pa", name="xhi")
            xlo8 = pb.tile([128, KD * BS], F8E4, tag="pb", name="xlo8")
            w1_pf = {}
            for m in range(3):
                w1_pf[m] = pw.tile([128, KD * 128], F8E4, tag="w", name=f"w1_{m}")
            for c in range(XCH):
                nc.scalar.dma_start(
                    w1_pf[0][:, c * KC * 128:(c + 1) * KC * 128],
                    w1_d[:, c * KC * 128:(c + 1) * KC * 128])
                sl = slice(c * KC * BS, (c + 1) * KC * BS)
                nc.scalar.dma_start(xhi[:, sl], xhi_d[:, sl])
            nc.scalar.dma_start(w1_pf[1][:], w1_d[:, KD * 128:2 * KD * 128])
            for c in range(XCH):
                sl = slice(c * KC * BS, (c + 1) * KC * BS)
                nc.scalar.dma_start(xlo8[:, sl], xlo_d[:, sl])
            nc.scalar.dma_start(w1_pf[2][:], w1_d[:, 2 * KD * 128:3 * KD * 128])
            xlo8v = xlo8[:].rearrange("p (k c) -> p k c", c=BS)

            # Wdr stream: fp8 +-1 DR-layout weights for L2/L3, 8-deep ring.
            # gens 0..15 = L2 m0..15, 16..31 = L3 m0..15.
            wdr_pf = {}

            def emit_wdr(gen):
                l, m = (2, gen) if gen < 16 else (3, gen - 16)
                w8t = pwdr.tile([128, KH * 128], F8E4, tag="wdr", name=f"wdr_{l}_{m}")
                nc.scalar.dma_start(w8t[:], wl_d[l][:, m * 2048:(m + 1) * 2048])
                wdr_pf[(l, m)] = w8t

            for gen in range(3):
                emit_wdr(gen)

            w4f = pconst.tile([128, C * KH], F16, tag="w4f")
            nc.scalar.dma_start(w4f[:], w4pk_d)

            parts = {}
            gchunk = {}     # (l, ci) -> allreduced pre-reduced stats tile
            stats = {}      # (l, ci) -> dict of stat tiles
            arouts = {}

            def emit_ar_fire(l, ci, do_sq):
                """n-pair-reduce sums, append sq cols -> DRAM -> AllReduce."""
                bd = _bounds(l)
                c0, c1 = bd[ci], bd[ci + 1]
                nm = c1 - c0
                w = 2 * nm if do_sq else nm
                red = pstat.tile([128, nm], F32, tag=f"red{l}{ci}", name=f"red{l}{ci}")
                nc.vector.tensor_reduce(
                    red[:],
                    parts[l][:, 2 * c0:2 * c1].rearrange("p (m n) -> p m n", n=2),
                    axis=mybir.AxisListType.X, op=ALU.add)
                arin = pdram.tile([128, w], F32, tag=f"arin{l}{ci}")
                arout = pdram.tile([128, w], F32, tag=f"arout{l}{ci}")
                nc.sync.dma_start(arin[:, 0:nm], red[:])
                if do_sq:
                    nc.sync.dma_start(arin[:, nm:w], parts[l][:, 32 + c0:32 + c1])
                nc.gpsimd.collective_compute(
                    "AllReduce", ALU.add, replica_groups=RG,
                    ins=[arin.opt()], outs=[arout.opt()])
                arouts[(l, ci)] = (arout, w)

            def emit_ar_land(l, ci):
                arout, w = arouts[(l, ci)]
                g_t = pstat.tile([128, w], F32, tag=f"g{l}{ci}", name=f"g{l}{ci}")
                nc.sync.dma_start(g_t[:], arout[:])
                gchunk[(l, ci)] = g_t

            def _st(l, ci, tag, nm):
                return pstat.tile([128, nm], F32, tag=f"{tag}{l}{ci}",
                                  name=f"{tag}{l}{ci}")

            def emit_stats_pre(l, ci, do_sq, fastl):
                """DVE-only stats from the pre-reduced AR result (safe to emit
                mid-loop: no ACT ops to block later drains)."""
                g_t = gchunk[(l, ci)]
                bd = _bounds(l)
                nm = bd[ci + 1] - bd[ci]
                m1 = _st(l, ci, "m1", nm)
                nc.vector.tensor_scalar_mul(m1[:], g_t[:, 0:nm], 1.0 / B)
                if fastl and not do_sq:
                    negm = _st(l, ci, "negm", nm)
                    nc.vector.tensor_scalar_mul(negm[:], g_t[:, 0:nm], -1.0 / B)
                    stats[(l, ci)] = dict(m1=m1, negm=negm, fast=True)
                    return
                msq, m1sq, v = (_st(l, ci, x, nm) for x in ("msq", "m1sq", "v"))
                nc.vector.tensor_scalar_mul(msq[:], g_t[:, nm:2 * nm], 1.0 / B)
                nc.vector.tensor_tensor(m1sq[:], m1[:], m1[:], op=ALU.mult)
                nc.vector.tensor_tensor(v[:], msq[:], m1sq[:], op=ALU.subtract)
                nc.vector.tensor_scalar_add(v[:], v[:], EPS)
                stats[(l, ci)] = dict(m1=m1, v=v, fast=False)

            def emit_stats_post(l, ci, fastl):
                """ACT sqrt + downstream scale/bias (emit after the layer's
                drains so the ACT queue never blocks on a pending AR)."""
                d = stats[(l, ci)]
                if d["fast"]:
                    return
                bd = _bounds(l)
                c0 = bd[ci]
                nm = bd[ci + 1] - c0
                gcol = cons[f"g{l}"][:, c0:c0 + nm]
                btcol = cons[f"bt{l}"][:, c0:c0 + nm]
                m1, v = d["m1"], d["v"]
                r, rp, mt, cc = (_st(l, ci, x, nm) for x in ("r", "rp", "mt", "c"))
                sq = _st(l, ci, "sq", nm)
                nc.scalar.activation(sq[:], v[:], ACT.Sqrt)
                nc.vector.reciprocal(r[:], sq[:])
                nc.vector.tensor_tensor(rp[:], gcol, r[:], op=ALU.mult)
                nc.vector.tensor_tensor(mt[:], m1[:], rp[:], op=ALU.mult)
                nc.vector.tensor_tensor(cc[:], btcol, mt[:], op=ALU.subtract)
                d.update(rp=rp, c=cc)
                if l < 3:
                    gi, u, u2, tthr, s, s2, sneg = (
                        _st(l, ci, x, nm)
                        for x in ("gi", "u", "u2", "tthr", "s", "s2", "sneg"))
                    nc.vector.reciprocal(gi[:], gcol)
                    nc.vector.tensor_tensor(u[:], btcol, gi[:], op=ALU.mult)
                    nc.vector.tensor_tensor(u2[:], u[:], sq[:], op=ALU.mult)
                    nc.vector.tensor_tensor(tthr[:], m1[:], u2[:], op=ALU.subtract)
                    nc.scalar.activation(s[:], gcol, ACT.Sign)
                    nc.vector.tensor_scalar_mul(s2[:], s[:], 2.0)
                    nc.vector.tensor_scalar_mul(sneg[:], s[:], -1.0)
                    d.update(tthr=tthr, s2=s2, sneg=sneg)

            def chunk_of(l, k):
                bd = _bounds(l)
                for ci in range(len(bd) - 1):
                    if k < bd[ci + 1]:
                        return ci, k - bd[ci]

            def sign_wave(l, dst3, h_t, krange, dve_only=False):
                """a[:, k, :] = sign-of-bn for k in krange; alternate ACT/DVE.
                dve_only for AR-end-gated chunks: keeps the ACT queue free of
                AR-gated ops so the next layer's drains + wdr DMA triggers
                (which ride the ACT queue in-order) are never blocked."""
                for k in krange:
                    ci, j = chunk_of(l, k)
                    s = stats[(l, ci)]
                    hsl = h_t[:, k * BS:(k + 1) * BS]
                    dst = dst3[:, k, :]
                    if k % 2 == 1 and not dve_only:
                        scale = 1.0 if s["fast"] else s["rp"][:, j:j + 1]
                        bias = s["negm"][:, j:j + 1] if s["fast"] else s["c"][:, j:j + 1]
                        nc.scalar.activation(dst, hsl, ACT.Sign, bias=bias, scale=scale)
                    elif s["fast"]:
                        # {0,1} encoding: weights are +-2 with bias correction
                        nc.vector.tensor_scalar(out=dst, in0=hsl,
                                                scalar1=s["m1"][:, j:j + 1],
                                                scalar2=None, op0=ALU.is_ge)
                    else:
                        thr = s["tthr"][:, j:j + 1]
                        bt_ = pscr.tile([128, BS], F16, tag="scr", name=f"sgb_{l}_{k}")
                        nc.vector.tensor_scalar(out=bt_[:], in0=hsl, scalar1=thr,
                                                scalar2=None, op0=ALU.is_ge)
                        nc.vector.tensor_scalar(out=dst, in0=bt_[:],
                                                scalar1=s["s2"][:, j:j + 1],
                                                scalar2=s["sneg"][:, j:j + 1],
                                                op0=ALU.mult, op1=ALU.add)

            def drain(l, m, n, ps, h_t, do_sq):
                hs = h_t[:, m * BS + n * 512: m * BS + n * 512 + 512]
                col = 2 * m + n
                nc.scalar.activation(hs, ps[:], ACT.Identity,
                                     bias=cons[f"b{l}"][:, m:m + 1], scale=1.0,
                                     accum_out=parts[l][:, col:col + 1])

            def msum(l, m, h_t, do_sq):
                """L3 sumsq: one fused DVE op (the ACT-Square+accum pair was
                2.8us/m-tile and made L3 ACT-bound)."""
                if not do_sq:
                    return
                hrow = h_t[:, m * BS:(m + 1) * BS]
                scr = pscr.tile([128, BS], F32, tag="scr", name=f"sq_{l}_{m}")
                nc.vector.tensor_tensor(scr[:], hrow, hrow, op=ALU.mult)
                nc.vector.tensor_reduce(
                    parts[l][:, 32 + m:32 + m + 1],
                    scr[:].rearrange("p (a b) -> p a b", a=1),
                    axis=mybir.AxisListType.X, op=ALU.add)

            def debug_out(src_ap, cast=False):
                if cast:
                    t = pscr.tile([128, BS], F32, tag="scr", name="dbgcast")
                    nc.vector.tensor_copy(t[:C, :], src_ap)
                    src_ap = t[:C, :]
                nc.sync.dma_start(out_d[:], src_ap)

            # ===================== Layer 1 =====================
            h1 = ph.tile([128, KH * BS], F32, tag="ph", name="h1")
            parts[1] = pstat.tile([128, 64], F32, tag="parts1", name="parts1")
            do_sq1 = not fast[0]
            bd1 = _bounds(1)

            def l1_mtile_alloc(m):
                st = {}
                st["wst"] = w1_pf.pop(m)
                st["w8lo"] = pw8.tile([128, KD * 128], F8E5, tag="w8",
                                      name=f"w8lo_{m}")
                st["w8lov"] = st["w8lo"][:].rearrange("p (k c) -> p k c", c=128)
                st["pss"] = [ppsum.tile([128, 512], F32, tag="ps",
                                        name=f"ps1_{m}_{n}") for n in range(NB)]
                return st

            def l1_chunk(st, c):
                wst, w8lo, w8lov, pss = (st["wst"], st["w8lo"], st["w8lov"],
                                         st["pss"])
                for k in range(c * KC, (c + 1) * KC):
                    lhsT = wst[:, k * 128:(k + 1) * 128]
                    for n in range(NB):
                        nc.tensor.matmul(
                            pss[n][:], lhsT,
                            xhi[:, k * BS + n * 512: k * BS + n * 512 + 512],
                            start=(k == 0), stop=False)
                nc.vector.tensor_scalar_mul(
                    w8lo[:, c * KC * 128:(c + 1) * KC * 128],
                    wst[:, c * KC * 128:(c + 1) * KC * 128], 2.0 ** -12)
                for t in range(c * KC // 2, (c + 1) * KC // 2):
                    lhsT = w8lov[:, 2 * t:2 * t + 2, :]
                    for n in range(NB):
                        nc.tensor.matmul(
                            pss[n][:], lhsT,
                            xlo8v[:, 2 * t:2 * t + 2, n * 512:n * 512 + 512],
                            start=False, stop=(t == KD // 2 - 1), perf_mode=DRM)

            def l1_tail(m, st):
                # W1 prefetch 3 ahead; Wdr gens 3..7 during m=8..12
                if m + 3 < KH and m + 3 not in w1_pf:
                    w1_pf[m + 3] = pw.tile([128, KD * 128], F8E4, tag="w",
                                           name=f"w1_{m + 3}")
                    nc.scalar.dma_start(
                        w1_pf[m + 3][:],
                        w1_d[:, (m + 3) * KD * 128:(m + 4) * KD * 128])
                if 8 <= m <= 12:
                    emit_wdr(m - 5)
                for n in range(NB):
                    drain(1, m, n, st["pss"][n], h1, do_sq1)
                msum(1, m, h1, do_sq1)
                for ci in range(len(bd1) - 1):
                    if m == bd1[ci + 1] - 1:
                        emit_ar_fire(1, ci, do_sq1)
                        if ci > 0:
                            emit_ar_land(1, ci - 1)

            # m=0,1 ride the incoming DMA wave: hi-passes chunk-paced on xhi,
            # lo-passes after (xlo streams behind xhi)
            st01 = {m: l1_mtile_alloc(m) for m in (0, 1)}
            for c in range(XCH):
                for m in (0, 1):
                    st = st01[m]
                    for k in range(c * KC, (c + 1) * KC):
                        lhsT = st["wst"][:, k * 128:(k + 1) * 128]
                        for n in range(NB):
                            nc.tensor.matmul(
                                st["pss"][n][:], lhsT,
                                xhi[:, k * BS + n * 512: k * BS + n * 512 + 512],
                                start=(k == 0), stop=False)
            for m in (0, 1):
                st = st01[m]
                nc.vector.tensor_scalar_mul(st["w8lo"][:], st["wst"][:], 2.0 ** -12)
            for c in range(XCH):
                for m in (0, 1):
                    st = st01[m]
                    for t in range(c * KC // 2, (c + 1) * KC // 2):
                        lhsT = st["w8lov"][:, 2 * t:2 * t + 2, :]
                        for n in range(NB):
                            nc.tensor.matmul(
                                st["pss"][n][:], lhsT,
                                xlo8v[:, 2 * t:2 * t + 2, n * 512:n * 512 + 512],
                                start=False, stop=(t == KD // 2 - 1),
                                perf_mode=DRM)
            for m in (0, 1):
                l1_tail(m, st01[m])
            for m in range(2, KH):
                st = l1_mtile_alloc(m)
                for c in range(XCH):
                    l1_chunk(st, c)
                l1_tail(m, st)
            emit_ar_land(1, len(bd1) - 2)

            if stage == 1:
                debug_out(h1[:C, :BS])

            a2 = pa.tile([128, KH, BS], F8E4, tag="pa", name="a2")
            nch1 = len(bd1) - 1
            for ci in range(nch1):
                emit_stats_pre(1, ci, do_sq1, fast[0])
                emit_stats_post(1, ci, fast[0])
                sign_wave(1, a2, h1, range(bd1[ci], bd1[ci + 1]),
                          dve_only=(ci == nch1 - 1))
            if stage == 2:
                debug_out(a2[:C, 0, :], cast=True)

            # ===================== Layers 2, 3 =====================
            def dense_dr(l, a_in):
                h_t = ph.tile([128, KH * BS], F32, tag="ph", name=f"h{l}")
                parts[l] = pstat.tile([128, 64], F32, tag=f"parts{l}", name=f"parts{l}")
                do_sq = (l == 3) or not fast[l - 1]
                bd = _bounds(l)
                # t-phases matching the PRODUCING layer's sign chunks
                pb_in = _bounds(l - 1)
                tph = [(pb_in[i] // 2, pb_in[i + 1] // 2) for i in range(len(pb_in) - 1)]
                for q in range(KH // 4):
                    ms = range(4 * q, 4 * q + 4)
                    pss = {m: [ppsum.tile([128, 512], F32, tag="ps",
                                          name=f"ps{l}_{m}_{n}") for n in range(NB)]
                           for m in ms}
                    w8 = {m: wdr_pf.pop((l, m)) for m in ms}
                    for ta, tb in tph:
                        for m in ms:
                            w8v = w8[m][:].rearrange("p (k c) -> p k c", c=128)
                            for t in range(ta, tb):
                                lhsT = w8v[:, 2 * t:2 * t + 2, :]
                                for n in range(NB):
                                    nc.tensor.matmul(
                                        pss[m][n][:], lhsT,
                                        a_in[:, 2 * t:2 * t + 2, n * 512:n * 512 + 512],
                                        start=(t == 0), stop=(t == KH // 2 - 1),
                                        perf_mode=DRM)
                    for m in ms:
                        gen = (l - 2) * 16 + m + 8
                        if gen < 32:
                            emit_wdr(gen)
                        for n in range(NB):
                            drain(l, m, n, pss[m][n], h_t, do_sq)
                        msum(l, m, h_t, do_sq)
                        for ci in range(len(bd) - 1):
                            if m == bd[ci + 1] - 1:
                                emit_ar_fire(l, ci, do_sq)
                                if ci > 0:
                                    emit_ar_land(l, ci - 1)
                emit_ar_land(l, len(bd) - 2)
                return h_t

            if stage >= 3:
                h2 = dense_dr(2, a2[:])
                a3 = pb.tile([128, KH, BS], F8E4, tag="pb", name="a3")
                bd2 = _bounds(2)
                for ci in range(len(bd2) - 1):
                    emit_stats_pre(2, ci, not fast[1], fast[1])
                    emit_stats_post(2, ci, fast[1])
                    sign_wave(2, a3, h2, range(bd2[ci], bd2[ci + 1]),
                              dve_only=(ci >= 1))
                if stage == 3:
                    debug_out(a3[:C, 0, :], cast=True)

            if stage >= 4:
                h3 = dense_dr(3, a3[:])
                # y3 = clip(bn3(h3), -1, 1) in fp16; L4 matmuls follow per k
                logits = plog.tile([16, BS], F32, tag="logits")
                ps4 = [ppsum.tile([128, 512], F32, tag="ps", name=f"ps4_{n}")
                       for n in range(NB)]
                y3dbg = None
                bd3 = _bounds(3)
                for ci in range(len(bd3) - 1):
                    emit_stats_pre(3, ci, True, False)
                for ci in range(len(bd3) - 1):
                    emit_stats_post(3, ci, False)
                    s = stats[(3, ci)]
                    for k in range(bd3[ci], bd3[ci + 1]):
                        j = k - bd3[ci]
                        scr = pscr.tile([128, BS], F32, tag="scr", name=f"y3s_{k}")
                        if k % 4 == 3:
                            # full-DVE path: offloads the serial ACT chain
                            nc.vector.tensor_scalar(
                                out=scr[:], in0=h3[:, k * BS:(k + 1) * BS],
                                scalar1=s["rp"][:, j:j + 1],
                                scalar2=s["c"][:, j:j + 1],
                                op0=ALU.mult, op1=ALU.add)
                        else:
                            nc.scalar.activation(scr[:],
                                                 h3[:, k * BS:(k + 1) * BS],
                                                 ACT.Identity,
                                                 bias=s["c"][:, j:j + 1],
                                                 scale=s["rp"][:, j:j + 1])
                        y3k = py3.tile([128, BS], F16, tag="y3", name=f"y3_{k}")
                        nc.vector.tensor_scalar(out=y3k[:], in0=scr[:],
                                                scalar1=-1.0, scalar2=1.0,
                                                op0=ALU.max, op1=ALU.min)
                        if k == 0:
                            y3dbg = y3k
                        if stage >= 5:
                            for n in range(NB):
                                nc.tensor.matmul(
                                    ps4[n][:C, :], w4f[:, k * C:(k + 1) * C],
                                    y3k[:, n * 512:(n + 1) * 512],
                                    start=(k == 0), stop=(k == KH - 1))
                if stage == 4:
                    debug_out(y3dbg[:C, :], cast=True)

            if stage >= 5:
                # ===== logits + log-softmax, 4 chunks of 256 cols =====
                for qq in range(4):
                    bank = ps4[qq // 2]
                    bsl = slice((qq % 2) * 256, (qq % 2) * 256 + 256)
                    qsl = slice(qq * 256, (qq + 1) * 256)
                    # logits on DVE (PSUM read) in parallel with exp on ACT
                    nc.vector.tensor_scalar(out=logits[:C, qsl], in0=bank[:C, bsl],
                                            scalar1=b4s[:C, :], scalar2=None,
                                            op0=ALU.add)
                    e_q = ptail.tile([16, 256], F32, tag="tl", name=f"e_{qq}")
                    nc.scalar.activation(e_q[:C, :], bank[:C, bsl], ACT.Exp,
                                         bias=b4s[:C, :], scale=1.0)
                    ps5 = ppsum.tile([128, 256], F32, tag="ps", name=f"ps5_{qq}")
                    nc.tensor.matmul(ps5[:1, :], ones10[:C, :], e_q[:C, :],
                                     start=True, stop=True)
                    lse_q = ptail.tile([16, 256], F32, tag="tl", name=f"lse_{qq}")
                    nc.scalar.activation(lse_q[:1, :], ps5[:1, :], ACT.Ln)
                    ps6 = ppsum.tile([128, 256], F32, tag="ps", name=f"ps6_{qq}")
                    nc.tensor.matmul(ps6[:C, :], onesC[:1, :C], lse_q[:1, :],
                                     start=True, stop=True)
                    outs_q = ptail.tile([16, 256], F32, tag="tl", name=f"o_{qq}")
                    nc.vector.tensor_tensor(outs_q[:C, :], logits[:C, qsl],
                                            ps6[:C, :], op=ALU.subtract)
                    if stage >= 6:
                        nc.sync.dma_start(out_d[:, qsl], outs_q[:C, :])
                if stage == 5:
                    debug_out(logits[:C, :])

    nc.compile()
    return nc


def _prep_inputs(x, W1, b1, g1, bt1, W2, b2, g2, bt2, W3, b3, g3, bt3, W4, b4):
    """Host-side sharding + layout prep (sign, fp8 cast, p-major packing)."""
    def as32(a):
        return np.ascontiguousarray(np.asarray(a, dtype=np.float32))

    f8 = ml_dtypes.float8_e4m3

    def sgn(w):
        return np.where(np.asarray(w) >= 0, np.float32(1.0), np.float32(-1.0))

    def pack_w(w, kt, uks=()):
        # [H_out, K] -> [128, (H_out/128) * K] with per-m-tile p-major blocks.
        # uks: k-blocks whose activations come {0,1}-encoded -> weights +-2.
        s = sgn(w).reshape(-1, 128, kt, 128)            # [m, c, k, p]
        if uks:
            s[:, :, sorted(uks), :] *= 2.0
        s = s.transpose(0, 3, 2, 1).reshape(s.shape[0], 128, kt * 128)
        return np.ascontiguousarray(
            s.transpose(1, 0, 2).reshape(128, -1)).astype(f8)

    def ok(g, bt):
        g, bt = np.asarray(g), np.asarray(bt)
        return bool(not np.any(bt) and np.all(g > 0))

    def ucorr(w, uks):
        # bias correction: -sum over u-encoded k-blocks of sign(w)
        if not uks:
            return 0.0
        s = sgn(w).reshape(w.shape[0], -1, 128)
        return s[:, sorted(uks), :].sum(axis=(1, 2))

    x = as32(x)
    u2 = _u_ks(1) if ok(g1, bt1) else set()
    u3 = _u_ks(2) if ok(g2, bt2) else set()
    W2, W3, b2, b3 = as32(W2), as32(W3), as32(b2), as32(b3)
    shared = {
        "w1pk": pack_w(as32(W1), KD),
        "w2pk": pack_w(W2, KH, u2),
        "w3pk": pack_w(W3, KH, u3),
    }
    b2 = b2 - ucorr(W2, u2)
    b3 = b3 - ucorr(W3, u3)
    cvecs = (b1, g1, bt1, b2, g2, bt2, b3, g3, bt3)
    cpk = np.empty((128, KH * len(cvecs)), np.float32)
    for i, v in enumerate(cvecs):
        cpk[:, i * KH:(i + 1) * KH] = as32(v).reshape(KH, 128).T
    shared["cpk"] = cpk
    w4T = np.ascontiguousarray(as32(W4).T)          # [H, C]
    w4pk = np.empty((128, C * KH), np.float16)
    for k in range(KH):
        w4pk[:, k * C:(k + 1) * C] = w4T[k * 128:(k + 1) * 128, :].astype(np.float16)
    shared["w4pk"] = w4pk
    b4p = np.zeros((16, 1), np.float32)
    b4p[:C, 0] = as32(b4).reshape(-1)
    shared["c_b4"] = b4p

    in_maps = []
    for cr in range(NCORES):
        xT = np.ascontiguousarray(x[cr * BS:(cr + 1) * BS].T)     # [D, BS]
        hi = xT.astype(np.float16)
        lo8 = ((xT - hi.astype(np.float32)) * 4096.0).astype(f8)
        # p-major pack: [D, BS] -> [128, KD*BS]
        hi_pk = np.ascontiguousarray(
            hi.reshape(KD, 128, BS).transpose(1, 0, 2).reshape(128, KD * BS))
        lo_pk = np.ascontiguousarray(
            lo8.reshape(KD, 128, BS).transpose(1, 0, 2).reshape(128, KD * BS))
        m = dict(shared)
        m["xT_hi"] = hi_pk
        m["xT_lo8"] = lo_pk
        in_maps.append(m)
    return in_maps


def _fast_flags(inputs):
    """Mean-only BN boundary valid when beta==0 and gamma>0."""
    def ok(g, bt):
        g, bt = np.asarray(g), np.asarray(bt)
        return bool(not np.any(bt) and np.all(g > 0))

    return (ok(inputs["g1"], inputs["bt1"]), ok(inputs["g2"], inputs["bt2"]))


def kernel(**inputs) -> np.ndarray:
    from concourse.bass_utils import run_bass_kernel_spmd

    fast = _fast_flags(inputs)
    if _CACHE.get("fast") != fast:
        _CACHE["nc"] = _build(fast=fast)
        _CACHE["fast"] = fast
    nc = _CACHE["nc"]
    in_maps = _prep_inputs(**inputs)
    res = run_bass_kernel_spmd(nc, in_maps, list(range(NCORES)))
    out = np.concatenate([res.results[c]["outT"].T for c in range(NCORES)], axis=0)
    return out.astype(np.float32)


# revision 20
# speedup vs baseline: 1.0700x; 1.0149x over previous
"""Trainium2 Bass kernel for nn_BinarizedCifar10MLP — v3.

Data-parallel over batch (8192/8 = 1024 rows/core), feature-major layout.

vs v2 (573us):
  - All weight signing moved to the HOST: W1 ships as fp8e4 +-1 (6.3MB,
    was 12.6MB bf16 + on-device sign), W2/W3 ship as fp8e4 +-1 in DR
    layout (no bf16 read + sign + DRAM round-trip prepass at all).
  - DMA queue discipline: bulk loads (x, W1, Wdr) ride the ACT hwdge
    queue; the sync queue carries only AR traffic + consts + final out.
    W1 m0 is FIRST on the queue (v2 had it behind all 9.4MB of x -> 43us
    PE stall at start); x chunks interleave with W1 m0 sub-tiles and the
    L1 m-loop consumes x chunk-by-chunk, so the PE rides the DMA wave.
  - 3-chunk BN-stat AllReduce for L2/L3 (m 0-9 / 10-13 / 14-15): AR-A
    fires at ~60% of the (short) layer instead of 87%, landing before
    the layer ends; quad-grouped phase-major matmul emission gives the
    PE a 4-m-tile runway on already-signed k-tiles while the tail AR
    lands. Stat sums are n-pair-reduced BEFORE the AR (half payload).
  - log-softmax tail in 4 chunks of 256 cols, exp computed straight
    from PSUM in parallel with the logits drain (DVE reads PSUM).
"""

import sys

sys.path.insert(0, "/opt/trn_rl_repo")

import numpy as np
import ml_dtypes

B, D, H, C = 8192, 3 * 32 * 32, 2048, 10
EPS = 1e-5
NCORES = 8
BS = B // NCORES          # 1024 batch rows per core
KD = D // 128             # 24 k-tiles over input dim
KH = H // 128             # 16 k-tiles over hidden dim
NB = BS // 512            # 2 free-dim chunks of 512
CHK = {1: (14, 16), 2: (10, 16), 3: (10, 16)}  # AR chunk end bounds
XCH = 6                   # x DMA chunks (4 k-tiles each)
KC = KD // XCH            # 6 k-tiles per x chunk

_CACHE = {}


def _bounds(l):
    return (0,) + CHK[l]


def _u_ks(l_prod):
    """k-tiles of the layer-l_prod sign output that are {0,1}-encoded (single
    DVE is_ge op); the consumer weights are host-scaled to +-2 on those blocks
    with the -rowsum(sign W) correction folded into the bias."""
    bd = _bounds(l_prod)
    nch = len(bd) - 1
    dve = {nch - 1} if l_prod == 1 else set(range(1, nch))
    ks = set()
    for ci in range(nch):
        for k in range(bd[ci], bd[ci + 1]):
            if ci in dve or k % 2 == 0:
                ks.add(k)
    return ks


def _build(stage=7, fast=(False, False)):
    import concourse.bacc as bacc
    import concourse.mybir as mybir
    import concourse.tile as tile

    F32 = mybir.dt.float32
    F16 = mybir.dt.float16
    F8E4 = mybir.dt.float8e4
    F8E5 = mybir.dt.float8e5
    DRM = mybir.MatmulPerfMode.DoubleRow
    ACT = mybir.ActivationFunctionType
    ALU = mybir.AluOpType
    RG = [list(range(NCORES))]

    nc = bacc.Bacc("TRN2", target_bir_lowering=False, debug=False, num_devices=NCORES)

    # ---- I/O ----
    xhi_d = nc.dram_tensor("xT_hi", [128, KD * BS], F16, kind="ExternalInput").ap()
    xlo_d = nc.dram_tensor("xT_lo8", [128, KD * BS], F8E4, kind="ExternalInput").ap()
    w1_d = nc.dram_tensor("w1pk", [128, KH * KD * 128], F8E4, kind="ExternalInput").ap()
    w2_d = nc.dram_tensor("w2pk", [128, KH * KH * 128], F8E4, kind="ExternalInput").ap()
    w3_d = nc.dram_tensor("w3pk", [128, KH * KH * 128], F8E4, kind="ExternalInput").ap()
    CNAMES = ("b1", "g1", "bt1", "b2", "g2", "bt2", "b3", "g3", "bt3")
    cpk_d = nc.dram_tensor("cpk", [128, KH * len(CNAMES)], F32, kind="ExternalInput").ap()
    w4pk_d = nc.dram_tensor("w4pk", [128, C * KH], F16, kind="ExternalInput").ap()
    b4_d = nc.dram_tensor("c_b4", [16, 1], F32, kind="ExternalInput").ap()
    out_d = nc.dram_tensor("outT", [C, BS], F32, kind="ExternalOutput").ap()

    wl_d = {2: w2_d, 3: w3_d}

    with tile.TileContext(nc) as tc:
        with (
            tc.tile_pool(name="pconst", bufs=1) as pconst,
            tc.tile_pool(name="pstat", bufs=1) as pstat,
            tc.tile_pool(name="plog", bufs=1) as plog,
            tc.tile_pool(name="ptail", bufs=4) as ptail,
            tc.tile_pool(name="pscr", bufs=3) as pscr,
            tc.tile_pool(name="pw", bufs=4) as pw,
            tc.tile_pool(name="pw8", bufs=2) as pw8,
            tc.tile_pool(name="pwdr", bufs=8) as pwdr,
            tc.tile_pool(name="py3", bufs=4) as py3,
            tc.tile_pool(name="ph", bufs=1) as ph,
            tc.tile_pool(name="pa", bufs=1) as pa,
            tc.tile_pool(name="pb", bufs=1) as pb,
            tc.tile_pool(name="ppsum", bufs=8, space="PSUM") as ppsum,
            tc.tile_pool(name="pdram", bufs=16, space="DRAM") as pdram,
        ):
            # ---- warmup AllReduce: absorbs ncfw first-collective staging ----
            wuin = pdram.tile([128, 4], F32, tag="wuin")
            wuout = pdram.tile([128, 4], F32, tag="wuout")
            wusrc = pstat.tile([128, 4], F32, tag="wusrc")
            nc.vector.memset(wusrc[:], 0.0)
            nc.sync.dma_start(wuin[:], wusrc[:])
            nc.gpsimd.collective_compute(
                "AllReduce", ALU.add, replica_groups=RG,
                ins=[wuin.opt()], outs=[wuout.opt()])

            # ---- constants (sync queue; small, land early) ----
            cpk = pconst.tile([128, KH * len(CNAMES)], F32, tag="cpk")
            nc.sync.dma_start(cpk[:], cpk_d)
            cons = {name: cpk[:, i * KH:(i + 1) * KH] for i, name in enumerate(CNAMES)}
            b4s = pconst.tile([16, 1], F32, tag="b4")
            nc.sync.dma_start(b4s[:], b4_d)
            ones10 = pconst.tile([16, 1], F32, tag="ones10")
            nc.vector.memset(ones10[:], 1.0)
            onesC = pconst.tile([1, 16], F32, tag="onesC")
            nc.vector.memset(onesC[:], 1.0)

            # ---- bulk loads: scalar (ACT hwdge) queue ----
            # W1 m0 interleaved chunk-wise with x so the PE starts ~5us in.
            xhi = pa.tile([128, KD * BS], F16, tag="pa", name="xhi")
            xlo8 = pb.tile([128, KD * BS], F8E4, tag="pb", name="xlo8")
            w1_pf = {}
            for m in range(3):
                w1_pf[m] = pw.tile([128, KD * 128], F8E4, tag="w", name=f"w1_{m}")
            for c in range(XCH):
                nc.scalar.dma_start(
                    w1_pf[0][:, c * KC * 128:(c + 1) * KC * 128],
                    w1_d[:, c * KC * 128:(c + 1) * KC * 128])
                sl = slice(c * KC * BS, (c + 1) * KC * BS)
                nc.scalar.dma_start(xhi[:, sl], xhi_d[:, sl])
            nc.scalar.dma_start(w1_pf[1][:], w1_d[:, KD * 128:2 * KD * 128])
            for c in range(XCH):
                sl = slice(c * KC * BS, (c + 1) * KC * BS)
                nc.scalar.dma_start(xlo8[:, sl], xlo_d[:, sl])
            nc.scalar.dma_start(w1_pf[2][:], w1_d[:, 2 * KD * 128:3 * KD * 128])
            xlo8v = xlo8[:].rearrange("p (k c) -> p k c", c=BS)

            # Wdr stream: fp8 +-1 DR-layout weights for L2/L3, 8-deep ring.
            # gens 0..15 = L2 m0..15, 16..31 = L3 m0..15.
            wdr_pf = {}

            def emit_wdr(gen):
                l, m = (2, gen) if gen < 16 else (3, gen - 16)
                w8t = pwdr.tile([128, KH * 128], F8E4, tag="wdr", name=f"wdr_{l}_{m}")
                nc.scalar.dma_start(w8t[:], wl_d[l][:, m * 2048:(m + 1) * 2048])
                wdr_pf[(l, m)] = w8t

            for gen in range(3):
                emit_wdr(gen)

            w4f = pconst.tile([128, C * KH], F16, tag="w4f")
            nc.scalar.dma_start(w4f[:], w4pk_d)

            parts = {}
            gchunk = {}     # (l, ci) -> allreduced pre-reduced stats tile
            stats = {}      # (l, ci) -> dict of stat tiles
            arouts = {}

            def emit_ar_fire(l, ci, do_sq):
                """n-pair-reduce sums, append sq cols -> DRAM -> AllReduce."""
                bd = _bounds(l)
                c0, c1 = bd[ci], bd[ci + 1]
                nm = c1 - c0
                w = 2 * nm if do_sq else nm
                red = pstat.tile([128, nm], F32, tag=f"red{l}{ci}", name=f"red{l}{ci}")
                nc.vector.tensor_reduce(
                    red[:],
                    parts[l][:, 2 * c0:2 * c1].rearrange("p (m n) -> p m n", n=2),
                    axis=mybir.AxisListType.X, op=ALU.add)
                arin = pdram.tile([128, w], F32, tag=f"arin{l}{ci}")
                arout = pdram.tile([128, w], F32, tag=f"arout{l}{ci}")
                nc.sync.dma_start(arin[:, 0:nm], red[:])
                if do_sq:
                    nc.sync.dma_start(arin[:, nm:w], parts[l][:, 32 + c0:32 + c1])
                nc.gpsimd.collective_compute(
                    "AllReduce", ALU.add, replica_groups=RG,
                    ins=[arin.opt()], outs=[arout.opt()])
                arouts[(l, ci)] = (arout, w)

            def emit_ar_land(l, ci):
                arout, w = arouts[(l, ci)]
                g_t = pstat.tile([128, w], F32, tag=f"g{l}{ci}", name=f"g{l}{ci}")
                nc.sync.dma_start(g_t[:], arout[:])
                gchunk[(l, ci)] = g_t

            def _st(l, ci, tag, nm):
                return pstat.tile([128, nm], F32, tag=f"{tag}{l}{ci}",
                                  name=f"{tag}{l}{ci}")

            def emit_stats_pre(l, ci, do_sq, fastl):
                """DVE-only stats from the pre-reduced AR result (safe to emit
                mid-loop: no ACT ops to block later drains)."""
                g_t = gchunk[(l, ci)]
                bd = _bounds(l)
                nm = bd[ci + 1] - bd[ci]
                m1 = _st(l, ci, "m1", nm)
                nc.vector.tensor_scalar_mul(m1[:], g_t[:, 0:nm], 1.0 / B)
                if fastl and not do_sq:
                    negm = _st(l, ci, "negm", nm)
                    nc.vector.tensor_scalar_mul(negm[:], g_t[:, 0:nm], -1.0 / B)
                    stats[(l, ci)] = dict(m1=m1, negm=negm, fast=True)
                    return
                msq, m1sq, v = (_st(l, ci, x, nm) for x in ("msq", "m1sq", "v"))
                nc.vector.tensor_scalar_mul(msq[:], g_t[:, nm:2 * nm], 1.0 / B)
                nc.vector.tensor_tensor(m1sq[:], m1[:], m1[:], op=ALU.mult)
                nc.vector.tensor_tensor(v[:], msq[:], m1sq[:], op=ALU.subtract)
                nc.vector.tensor_scalar_add(v[:], v[:], EPS)
                stats[(l, ci)] = dict(m1=m1, v=v, fast=False)

            def emit_stats_post(l, ci, fastl):
                """ACT sqrt + downstream scale/bias (emit after the layer's
                drains so the ACT queue never blocks on a pending AR)."""
                d = stats[(l, ci)]
                if d["fast"]:
                    return
                bd = _bounds(l)
                c0 = bd[ci]
                nm = bd[ci + 1] - c0
                gcol = cons[f"g{l}"][:, c0:c0 + nm]
                btcol = cons[f"bt{l}"][:, c0:c0 + nm]
                m1, v = d["m1"], d["v"]
                r, rp, mt, cc = (_st(l, ci, x, nm) for x in ("r", "rp", "mt", "c"))
                sq = _st(l, ci, "sq", nm)
                nc.scalar.activation(sq[:], v[:], ACT.Sqrt)
                nc.vector.reciprocal(r[:], sq[:])
                nc.vector.tensor_tensor(rp[:], gcol, r[:], op=ALU.mult)
                nc.vector.tensor_tensor(mt[:], m1[:], rp[:], op=ALU.mult)
                nc.vector.tensor_tensor(cc[:], btcol, mt[:], op=ALU.subtract)
                d.update(rp=rp, c=cc)
                if l < 3:
                    gi, u, u2, tthr, s, s2, sneg = (
                        _st(l, ci, x, nm)
                        for x in ("gi", "u", "u2", "tthr", "s", "s2", "sneg"))
                    nc.vector.reciprocal(gi[:], gcol)
                    nc.vector.tensor_tensor(u[:], btcol, gi[:], op=ALU.mult)
                    nc.vector.tensor_tensor(u2[:], u[:], sq[:], op=ALU.mult)
                    nc.vector.tensor_tensor(tthr[:], m1[:], u2[:], op=ALU.subtract)
                    nc.scalar.activation(s[:], gcol, ACT.Sign)
                    nc.vector.tensor_scalar_mul(s2[:], s[:], 2.0)
                    nc.vector.tensor_scalar_mul(sneg[:], s[:], -1.0)
                    d.update(tthr=tthr, s2=s2, sneg=sneg)

            def chunk_of(l, k):
                bd = _bounds(l)
                for ci in range(len(bd) - 1):
                    if k < bd[ci + 1]:
                        return ci, k - bd[ci]

            def sign_wave(l, dst3, h_t, krange, dve_only=False):
                """a[:, k, :] = sign-of-bn for k in krange; alternate ACT/DVE.
                dve_only for AR-end-gated chunks: keeps the ACT queue free of
                AR-gated ops so the next layer's drains + wdr DMA triggers
                (which ride the ACT queue in-order) are never blocked."""
                for k in krange:
                    ci, j = chunk_of(l, k)
                    s = stats[(l, ci)]
                    hsl = h_t[:, k * BS:(k + 1) * BS]
                    dst = dst3[:, k, :]
                    if k % 2 == 1 and not dve_only:
                        scale = 1.0 if s["fast"] else s["rp"][:, j:j + 1]
                        bias = s["negm"][:, j:j + 1] if s["fast"] else s["c"][:, j:j + 1]
                        nc.scalar.activation(dst, hsl, ACT.Sign, bias=bias, scale=scale)
                    elif s["fast"]:
                        # {0,1} encoding: weights are +-2 with bias correction
                        nc.vector.tensor_scalar(out=dst, in0=hsl,
                                                scalar1=s["m1"][:, j:j + 1],
                                                scalar2=None, op0=ALU.is_ge)
                    else:
                        thr = s["tthr"][:, j:j + 1]
                        bt_ = pscr.tile([128, BS], F16, tag="scr", name=f"sgb_{l}_{k}")
                        nc.vector.tensor_scalar(out=bt_[:], in0=hsl, scalar1=thr,
                                                scalar2=None, op0=ALU.is_ge)
                        nc.vector.tensor_scalar(out=dst, in0=bt_[:],
                                                scalar1=s["s2"][:, j:j + 1],
                                                scalar2=s["sneg"][:, j:j + 1],
                                                op0=ALU.mult, op1=ALU.add)

            def drain(l, m, n, ps, h_t, do_sq):
                hs = h_t[:, m * BS + n * 512: m * BS + n * 512 + 512]
                col = 2 * m + n
                nc.scalar.activation(hs, ps[:], ACT.Identity,
                                     bias=cons[f"b{l}"][:, m:m + 1], scale=1.0,
                                     accum_out=parts[l][:, col:col + 1])

            def msum(l, m, h_t, do_sq):
                """L3 sumsq: one fused DVE op (the ACT-Square+accum pair was
                2.8us/m-tile and made L3 ACT-bound)."""
                if not do_sq:
                    return
                hrow = h_t[:, m * BS:(m + 1) * BS]
                scr = pscr.tile([128, BS], F32, tag="scr", name=f"sq_{l}_{m}")
                nc.vector.tensor_tensor(scr[:], hrow, hrow, op=ALU.mult)
                nc.vector.tensor_reduce(
                    parts[l][:, 32 + m:32 + m + 1],
                    scr[:].rearrange("p (a b) -> p a b", a=1),
                    axis=mybir.AxisListType.X, op=ALU.add)

            def debug_out(src_ap, cast=False):
                if cast:
                    t = pscr.tile([128, BS], F32, tag="scr", name="dbgcast")
                    nc.vector.tensor_copy(t[:C, :], src_ap)
                    src_ap = t[:C, :]
                nc.sync.dma_start(out_d[:], src_ap)

            # ===================== Layer 1 =====================
            h1 = ph.tile([128, KH * BS], F32, tag="ph", name="h1")
            parts[1] = pstat.tile([128, 64], F32, tag="parts1", name="parts1")
            do_sq1 = not fast[0]
            bd1 = _bounds(1)

            def l1_mtile_alloc(m):
                st = {}
                st["wst"] = w1_pf.pop(m)
                st["w8lo"] = pw8.tile([128, KD * 128], F8E5, tag="w8",
                                      name=f"w8lo_{m}")
                st["w8lov"] = st["w8lo"][:].rearrange("p (k c) -> p k c", c=128)
                st["pss"] = [ppsum.tile([128, 512], F32, tag="ps",
                                        name=f"ps1_{m}_{n}") for n in range(NB)]
                return st

            def l1_chunk(st, c):
                wst, w8lo, w8lov, pss = (st["wst"], st["w8lo"], st["w8lov"],
                                         st["pss"])
                for k in range(c * KC, (c + 1) * KC):
                    lhsT = wst[:, k * 128:(k + 1) * 128]
                    for n in range(NB):
                        nc.tensor.matmul(
                            pss[n][:], lhsT,
                            xhi[:, k * BS + n * 512: k * BS + n * 512 + 512],
                            start=(k == 0), stop=False)
                nc.vector.tensor_scalar_mul(
                    w8lo[:, c * KC * 128:(c + 1) * KC * 128],
                    wst[:, c * KC * 128:(c + 1) * KC * 128], 2.0 ** -12)
                for t in range(c * KC // 2, (c + 1) * KC // 2):
                    lhsT = w8lov[:, 2 * t:2 * t + 2, :]
                    for n in range(NB):
                        nc.tensor.matmul(
                            pss[n][:], lhsT,
                            xlo8v[:, 2 * t:2 * t + 2, n * 512:n * 512 + 512],
                            start=False, stop=(t == KD // 2 - 1), perf_mode=DRM)

            def l1_tail(m, st):
                # W1 prefetch 3 ahead; Wdr gens 3..7 during m=8..12
                if m + 3 < KH and m + 3 not in w1_pf:
                    w1_pf[m + 3] = pw.tile([128, KD * 128], F8E4, tag="w",
                                           name=f"w1_{m + 3}")
                    nc.scalar.dma_start(
                        w1_pf[m + 3][:],
                        w1_d[:, (m + 3) * KD * 128:(m + 4) * KD * 128])
                if 8 <= m <= 12:
                    emit_wdr(m - 5)
                for n in range(NB):
                    drain(1, m, n, st["pss"][n], h1, do_sq1)
                msum(1, m, h1, do_sq1)
                for ci in range(len(bd1) - 1):
                    if m == bd1[ci + 1] - 1:
                        emit_ar_fire(1, ci, do_sq1)
                        if ci > 0:
                            emit_ar_land(1, ci - 1)

            # m=0,1 ride the incoming DMA wave: hi-passes chunk-paced on xhi,
            # lo-passes after (xlo streams behind xhi)
            st01 = {m: l1_mtile_alloc(m) for m in (0, 1)}
            for c in range(XCH):
                for m in (0, 1):
                    st = st01[m]
                    for k in range(c * KC, (c + 1) * KC):
                        lhsT = st["wst"][:, k * 128:(k + 1) * 128]
                        for n in range(NB):
                            nc.tensor.matmul(
                                st["pss"][n][:], lhsT,
                                xhi[:, k * BS + n * 512: k * BS + n * 512 + 512],
                                start=(k == 0), stop=False)
            for m in (0, 1):
                st = st01[m]
                nc.vector.tensor_scalar_mul(st["w8lo"][:], st["wst"][:], 2.0 ** -12)
            for c in range(XCH):
                for m in (0, 1):
                    st = st01[m]
                    for t in range(c * KC // 2, (c + 1) * KC // 2):
                        lhsT = st["w8lov"][:, 2 * t:2 * t + 2, :]
                        for n in range(NB):
                            nc.tensor.matmul(
                                st["pss"][n][:], lhsT,
                                xlo8v[:, 2 * t:2 * t + 2, n * 512:n * 512 + 512],
                                start=False, stop=(t == KD // 2 - 1),
                                perf_mode=DRM)
            for m in (0, 1):
                l1_tail(m, st01[m])
            for m in range(2, KH):
                st = l1_mtile_alloc(m)
                for c in range(XCH):
                    l1_chunk(st, c)
                l1_tail(m, st)
            emit_ar_land(1, len(bd1) - 2)

            if stage == 1:
                debug_out(h1[:C, :BS])

            a2 = pa.tile([128, KH, BS], F8E4, tag="pa", name="a2")
            nch1 = len(bd1) - 1
            for ci in range(nch1):
                emit_stats_pre(1, ci, do_sq1, fast[0])
                emit_stats_post(1, ci, fast[0])
                sign_wave(1, a2, h1, range(bd1[ci], bd1[ci + 1]),
                          dve_only=(ci == nch1 - 1))
            if stage == 2:
                debug_out(a2[:C, 0, :], cast=True)

            # ===================== Layers 2, 3 =====================
            def dense_dr(l, a_in):
                h_t = ph.tile([128, KH * BS], F32, tag="ph", name=f"h{l}")
                parts[l] = pstat.tile([128, 64], F32, tag=f"parts{l}", name=f"parts{l}")
                do_sq = (l == 3) or not fast[l - 1]
                bd = _bounds(l)
                # t-phases matching the PRODUCING layer's sign chunks
                pb_in = _bounds(l - 1)
                tph = [(pb_in[i] // 2, pb_in[i + 1] // 2) for i in range(len(pb_in) - 1)]
                for q in range(KH // 4):
                    ms = range(4 * q, 4 * q + 4)
                    pss = {m: [ppsum.tile([128, 512], F32, tag="ps",
                                          name=f"ps{l}_{m}_{n}") for n in range(NB)]
                           for m in ms}
                    w8 = {m: wdr_pf.pop((l, m)) for m in ms}
                    for ta, tb in tph:
                        for m in ms:
                            w8v = w8[m][:].rearrange("p (k c) -> p k c", c=128)
                            for t in range(ta, tb):
                                lhsT = w8v[:, 2 * t:2 * t + 2, :]
                                for n in range(NB):
                                    nc.tensor.matmul(
                                        pss[m][n][:], lhsT,
                                        a_in[:, 2 * t:2 * t + 2, n * 512:n * 512 + 512],
                                        start=(t == 0), stop=(t == KH // 2 - 1),
                                        perf_mode=DRM)
                    for m in ms:
                        gen = (l - 2) * 16 + m + 8
                        if gen < 32:
                            emit_wdr(gen)
                        for n in range(NB):
                            drain(l, m, n, pss[m][n], h_t, do_sq)
                        msum(l, m, h_t, do_sq)
                        for ci in range(len(bd) - 1):
                            if m == bd[ci + 1] - 1:
                                emit_ar_fire(l, ci, do_sq)
                                if ci > 0:
                                    emit_ar_land(l, ci - 1)
                emit_ar_land(l, len(bd) - 2)
                return h_t

            if stage >= 3:
                h2 = dense_dr(2, a2[:])
                a3 = pb.tile([128, KH, BS], F8E4, tag="pb", name="a3")
                bd2 = _bounds(2)
                for ci in range(len(bd2) - 1):
                    emit_stats_pre(2, ci, not fast[1], fast[1])
                    emit_stats_post(2, ci, fast[1])
                    sign_wave(2, a3, h2, range(bd2[ci], bd2[ci + 1]),
                              dve_only=(ci >= 1))
                if stage == 3:
                    debug_out(a3[:C, 0, :], cast=True)

            if stage >= 4:
                h3 = dense_dr(3, a3[:])
                # y3 = clip(bn3(h3), -1, 1) in fp16; L4 matmuls follow per k
                logits = plog.tile([16, BS], F32, tag="logits")
                ps4 = [ppsum.tile([128, 512], F32, tag="ps", name=f"ps4_{n}")
                       for n in range(NB)]
                y3dbg = None
                bd3 = _bounds(3)
                for ci in range(len(bd3) - 1):
                    emit_stats_pre(3, ci, True, False)
                for ci in range(len(bd3) - 1):
                    emit_stats_post(3, ci, False)
                    s = stats[(3, ci)]
                    for k in range(bd3[ci], bd3[ci + 1]):
                        j = k - bd3[ci]
                        scr = pscr.tile([128, BS], F32, tag="scr", name=f"y3s_{k}")
                        if k % 4 == 3:
                            # full-DVE path: offloads the serial ACT chain
                            nc.vector.tensor_scalar(
                                out=scr[:], in0=h3[:, k * BS:(k + 1) * BS],
                                scalar1=s["rp"][:, j:j + 1],
                                scalar2=s["c"][:, j:j + 1],
                                op0=ALU.mult, op1=ALU.add)
                        else:
                            nc.scalar.activation(scr[:],
                                                 h3[:, k * BS:(k + 1) * BS],
                                                 ACT.Identity,
                                                 bias=s["c"][:, j:j + 1],
                                                 scale=s["rp"][:, j:j + 1])
                        y3k = py3.tile([128, BS], F16, tag="y3", name=f"y3_{k}")
                        nc.vector.tensor_scalar(out=y3k[:], in0=scr[:],
                                                scalar1=-1.0, scalar2=1.0,
                                                op0=ALU.max, op1=ALU.min)
                        if k == 0:
                            y3dbg = y3k
                        if stage >= 5:
                            for n in range(NB):
                                nc.tensor.matmul(
                                    ps4[n][:C, :], w4f[:, k * C:(k + 1) * C],
                                    y3k[:, n * 512:(n + 1) * 512],
                                    start=(k == 0), stop=(k == KH - 1))
                if stage == 4:
                    debug_out(y3dbg[:C, :], cast=True)

            if stage >= 5:
                # ===== logits + log-softmax, 4 chunks of 256 cols =====
                for qq in range(2):
                    bank = ps4[qq]
                    qsl = slice(qq * 512, (qq + 1) * 512)
                    # logits on DVE (PSUM read) in parallel with exp on ACT
                    nc.vector.tensor_scalar(out=logits[:C, qsl], in0=bank[:C, :],
                                            scalar1=b4s[:C, :], scalar2=None,
                                            op0=ALU.add)
                    e_q = ptail.tile([16, 512], F32, tag="tl", name=f"e_{qq}")
                    nc.scalar.activation(e_q[:C, :], bank[:C, :], ACT.Exp,
                                         bias=b4s[:C, :], scale=1.0)
                    ps5 = ppsum.tile([128, 512], F32, tag="ps", name=f"ps5_{qq}")
                    nc.tensor.matmul(ps5[:1, :], ones10[:C, :], e_q[:C, :],
                                     start=True, stop=True)
                    lse_q = ptail.tile([16, 512], F32, tag="tl", name=f"lse_{qq}")
                    nc.scalar.activation(lse_q[:1, :], ps5[:1, :], ACT.Ln)
                    ps6 = ppsum.tile([128, 512], F32, tag="ps", name=f"ps6_{qq}")
                    nc.tensor.matmul(ps6[:C, :], onesC[:1, :C], lse_q[:1, :],
                                     start=True, stop=True)
                    outs_q = ptail.tile([16, 512], F32, tag="tl", name=f"o_{qq}")
                    nc.vector.tensor_tensor(outs_q[:C, :], logits[:C, qsl],
                                            ps6[:C, :], op=ALU.subtract)
                    if stage >= 6:
                        nc.sync.dma_start(out_d[:, qsl], outs_q[:C, :])
                if stage == 5:
                    debug_out(logits[:C, :])

    nc.compile()
    return nc


def _prep_inputs(x, W1, b1, g1, bt1, W2, b2, g2, bt2, W3, b3, g3, bt3, W4, b4):
    """Host-side sharding + layout prep (sign, fp8 cast, p-major packing)."""
    def as32(a):
        return np.ascontiguousarray(np.asarray(a, dtype=np.float32))

    f8 = ml_dtypes.float8_e4m3

    def sgn(w):
        return np.where(np.asarray(w) >= 0, np.float32(1.0), np.float32(-1.0))

    def pack_w(w, kt, uks=()):
        # [H_out, K] -> [128, (H_out/128) * K] with per-m-tile p-major blocks.
        # uks: k-blocks whose activations come {0,1}-encoded -> weights +-2.
        s = sgn(w).reshape(-1, 128, kt, 128)            # [m, c, k, p]
        if uks:
            s[:, :, sorted(uks), :] *= 2.0
        s = s.transpose(0, 3, 2, 1).reshape(s.shape[0], 128, kt * 128)
        return np.ascontiguousarray(
            s.transpose(1, 0, 2).reshape(128, -1)).astype(f8)

    def ok(g, bt):
        g, bt = np.asarray(g), np.asarray(bt)
        return bool(not np.any(bt) and np.all(g > 0))

    def ucorr(w, uks):
        # bias correction: -sum over u-encoded k-blocks of sign(w)
        if not uks:
            return 0.0
        s = sgn(w).reshape(w.shape[0], -1, 128)
        return s[:, sorted(uks), :].sum(axis=(1, 2))

    x = as32(x)
    u2 = _u_ks(1) if ok(g1, bt1) else set()
    u3 = _u_ks(2) if ok(g2, bt2) else set()
    W2, W3, b2, b3 = as32(W2), as32(W3), as32(b2), as32(b3)
    shared = {
        "w1pk": pack_w(as32(W1), KD),
        "w2pk": pack_w(W2, KH, u2),
        "w3pk": pack_w(W3, KH, u3),
    }
    b2 = b2 - ucorr(W2, u2)
    b3 = b3 - ucorr(W3, u3)
    cvecs = (b1, g1, bt1, b2, g2, bt2, b3, g3, bt3)
    cpk = np.empty((128, KH * len(cvecs)), np.float32)
    for i, v in enumerate(cvecs):
        cpk[:, i * KH:(i + 1) * KH] = as32(v).reshape(KH, 128).T
    shared["cpk"] = cpk
    w4T = np.ascontiguousarray(as32(W4).T)          # [H, C]
    w4pk = np.empty((128, C * KH), np.float16)
    for k in range(KH):
        w4pk[:, k * C:(k + 1) * C] = w4T[k * 128:(k + 1) * 128, :].astype(np.float16)
    shared["w4pk"] = w4pk
    b4p = np.zeros((16, 1), np.float32)
    b4p[:C, 0] = as32(b4).reshape(-1)
    shared["c_b4"] = b4p

    in_maps = []
    for cr in range(NCORES):
        xT = np.ascontiguousarray(x[cr * BS:(cr + 1) * BS].T)     # [D, BS]
        hi = xT.astype(np.float16)
        lo8 = ((xT - hi.astype(np.float32)) * 4096.0).astype(f8)
        # p-major pack: [D, BS] -> [128, KD*BS]
        hi_pk = np.ascontiguousarray(
            hi.reshape(KD, 128, BS).transpose(1, 0, 2).reshape(128, KD * BS))
        lo_pk = np.ascontiguousarray(
            lo8.reshape(KD, 128, BS).transpose(1, 0, 2).reshape(128, KD * BS))
        m = dict(shared)
        m["xT_hi"] = hi_pk
        m["xT_lo8"] = lo_pk
        in_maps.append(m)
    return in_maps


def _fast_flags(inputs):
    """Mean-only BN boundary valid when beta==0 and gamma>0."""
    def ok(g, bt):
        g, bt = np.asarray(g), np.asarray(bt)
        return bool(not np.any(bt) and np.all(g > 0))

    return (ok(inputs["g1"], inputs["bt1"]), ok(inputs["g2"], inputs["bt2"]))


def kernel(**inputs) -> np.ndarray:
    from concourse.bass_utils import run_bass_kernel_spmd

    fast = _fast_flags(inputs)
    if _CACHE.get("fast") != fast:
        _CACHE["nc"] = _build(fast=fast)
        _CACHE["fast"] = fast
    nc = _CACHE["nc"]
    in_maps = _prep_inputs(**inputs)
    res = run_bass_kernel_spmd(nc, in_maps, list(range(NCORES)))
    out = np.concatenate([res.results[c]["outT"].T for c in range(NCORES)], axis=0)
    return out.astype(np.float32)


# revision 21
# speedup vs baseline: 1.0718x; 1.0016x over previous
"""Trainium2 Bass kernel for nn_BinarizedCifar10MLP — v3.

Data-parallel over batch (8192/8 = 1024 rows/core), feature-major layout.

vs v2 (573us):
  - All weight signing moved to the HOST: W1 ships as fp8e4 +-1 (6.3MB,
    was 12.6MB bf16 + on-device sign), W2/W3 ship as fp8e4 +-1 in DR
    layout (no bf16 read + sign + DRAM round-trip prepass at all).
  - DMA queue discipline: bulk loads (x, W1, Wdr) ride the ACT hwdge
    queue; the sync queue carries only AR traffic + consts + final out.
    W1 m0 is FIRST on the queue (v2 had it behind all 9.4MB of x -> 43us
    PE stall at start); x chunks interleave with W1 m0 sub-tiles and the
    L1 m-loop consumes x chunk-by-chunk, so the PE rides the DMA wave.
  - 3-chunk BN-stat AllReduce for L2/L3 (m 0-9 / 10-13 / 14-15): AR-A
    fires at ~60% of the (short) layer instead of 87%, landing before
    the layer ends; quad-grouped phase-major matmul emission gives the
    PE a 4-m-tile runway on already-signed k-tiles while the tail AR
    lands. Stat sums are n-pair-reduced BEFORE the AR (half payload).
  - log-softmax tail in 4 chunks of 256 cols, exp computed straight
    from PSUM in parallel with the logits drain (DVE reads PSUM).
"""

import sys

sys.path.insert(0, "/opt/trn_rl_repo")

import numpy as np
import ml_dtypes

B, D, H, C = 8192, 3 * 32 * 32, 2048, 10
EPS = 1e-5
NCORES = 8
BS = B // NCORES          # 1024 batch rows per core
KD = D // 128             # 24 k-tiles over input dim
KH = H // 128             # 16 k-tiles over hidden dim
NB = BS // 512            # 2 free-dim chunks of 512
CHK = {1: (14, 16), 2: (10, 16), 3: (12, 16)}  # AR chunk end bounds
XCH = 6                   # x DMA chunks (4 k-tiles each)
KC = KD // XCH            # 6 k-tiles per x chunk

_CACHE = {}


def _bounds(l):
    return (0,) + CHK[l]


def _u_ks(l_prod):
    """k-tiles of the layer-l_prod sign output that are {0,1}-encoded (single
    DVE is_ge op); the consumer weights are host-scaled to +-2 on those blocks
    with the -rowsum(sign W) correction folded into the bias."""
    bd = _bounds(l_prod)
    nch = len(bd) - 1
    dve = {nch - 1} if l_prod == 1 else set(range(1, nch))
    ks = set()
    for ci in range(nch):
        for k in range(bd[ci], bd[ci + 1]):
            if ci in dve or k % 2 == 0:
                ks.add(k)
    return ks


def _build(stage=7, fast=(False, False)):
    import concourse.bacc as bacc
    import concourse.mybir as mybir
    import concourse.tile as tile

    F32 = mybir.dt.float32
    F16 = mybir.dt.float16
    F8E4 = mybir.dt.float8e4
    F8E5 = mybir.dt.float8e5
    DRM = mybir.MatmulPerfMode.DoubleRow
    ACT = mybir.ActivationFunctionType
    ALU = mybir.AluOpType
    RG = [list(range(NCORES))]

    nc = bacc.Bacc("TRN2", target_bir_lowering=False, debug=False, num_devices=NCORES)

    # ---- I/O ----
    xhi_d = nc.dram_tensor("xT_hi", [128, KD * BS], F16, kind="ExternalInput").ap()
    xlo_d = nc.dram_tensor("xT_lo8", [128, KD * BS], F8E4, kind="ExternalInput").ap()
    w1_d = nc.dram_tensor("w1pk", [128, KH * KD * 128], F8E4, kind="ExternalInput").ap()
    w2_d = nc.dram_tensor("w2pk", [128, KH * KH * 128], F8E4, kind="ExternalInput").ap()
    w3_d = nc.dram_tensor("w3pk", [128, KH * KH * 128], F8E4, kind="ExternalInput").ap()
    CNAMES = ("b1", "g1", "bt1", "b2", "g2", "bt2", "b3", "g3", "bt3")
    cpk_d = nc.dram_tensor("cpk", [128, KH * len(CNAMES)], F32, kind="ExternalInput").ap()
    w4pk_d = nc.dram_tensor("w4pk", [128, C * KH], F16, kind="ExternalInput").ap()
    b4_d = nc.dram_tensor("c_b4", [16, 1], F32, kind="ExternalInput").ap()
    out_d = nc.dram_tensor("outT", [C, BS], F32, kind="ExternalOutput").ap()

    wl_d = {2: w2_d, 3: w3_d}

    with tile.TileContext(nc) as tc:
        with (
            tc.tile_pool(name="pconst", bufs=1) as pconst,
            tc.tile_pool(name="pstat", bufs=1) as pstat,
            tc.tile_pool(name="plog", bufs=1) as plog,
            tc.tile_pool(name="ptail", bufs=4) as ptail,
            tc.tile_pool(name="pscr", bufs=3) as pscr,
            tc.tile_pool(name="pw", bufs=4) as pw,
            tc.tile_pool(name="pw8", bufs=2) as pw8,
            tc.tile_pool(name="pwdr", bufs=8) as pwdr,
            tc.tile_pool(name="py3", bufs=4) as py3,
            tc.tile_pool(name="ph", bufs=1) as ph,
            tc.tile_pool(name="pa", bufs=1) as pa,
            tc.tile_pool(name="pb", bufs=1) as pb,
            tc.tile_pool(name="ppsum", bufs=8, space="PSUM") as ppsum,
            tc.tile_pool(name="pdram", bufs=16, space="DRAM") as pdram,
        ):
            # ---- warmup AllReduce: absorbs ncfw first-collective staging ----
            wuin = pdram.tile([128, 4], F32, tag="wuin")
            wuout = pdram.tile([128, 4], F32, tag="wuout")
            wusrc = pstat.tile([128, 4], F32, tag="wusrc")
            nc.vector.memset(wusrc[:], 0.0)
            nc.sync.dma_start(wuin[:], wusrc[:])
            nc.gpsimd.collective_compute(
                "AllReduce", ALU.add, replica_groups=RG,
                ins=[wuin.opt()], outs=[wuout.opt()])

            # ---- constants (sync queue; small, land early) ----
            cpk = pconst.tile([128, KH * len(CNAMES)], F32, tag="cpk")
            nc.sync.dma_start(cpk[:], cpk_d)
            cons = {name: cpk[:, i * KH:(i + 1) * KH] for i, name in enumerate(CNAMES)}
            b4s = pconst.tile([16, 1], F32, tag="b4")
            nc.sync.dma_start(b4s[:], b4_d)
            ones10 = pconst.tile([16, 1], F32, tag="ones10")
            nc.vector.memset(ones10[:], 1.0)
            onesC = pconst.tile([1, 16], F32, tag="onesC")
            nc.vector.memset(onesC[:], 1.0)

            # ---- bulk loads: scalar (ACT hwdge) queue ----
            # W1 m0 interleaved chunk-wise with x so the PE starts ~5us in.
            xhi = pa.tile([128, KD * BS], F16, tag="pa", name="xhi")
            xlo8 = pb.tile([128, KD * BS], F8E4, tag="pb", name="xlo8")
            w1_pf = {}
            for m in range(3):
                w1_pf[m] = pw.tile([128, KD * 128], F8E4, tag="w", name=f"w1_{m}")
            for c in range(XCH):
                nc.scalar.dma_start(
                    w1_pf[0][:, c * KC * 128:(c + 1) * KC * 128],
                    w1_d[:, c * KC * 128:(c + 1) * KC * 128])
                sl = slice(c * KC * BS, (c + 1) * KC * BS)
                nc.scalar.dma_start(xhi[:, sl], xhi_d[:, sl])
            nc.scalar.dma_start(w1_pf[1][:], w1_d[:, KD * 128:2 * KD * 128])
            for c in range(XCH):
                sl = slice(c * KC * BS, (c + 1) * KC * BS)
                nc.scalar.dma_start(xlo8[:, sl], xlo_d[:, sl])
            nc.scalar.dma_start(w1_pf[2][:], w1_d[:, 2 * KD * 128:3 * KD * 128])
            xlo8v = xlo8[:].rearrange("p (k c) -> p k c", c=BS)

            # Wdr stream: fp8 +-1 DR-layout weights for L2/L3, 8-deep ring.
            # gens 0..15 = L2 m0..15, 16..31 = L3 m0..15.
            wdr_pf = {}

            def emit_wdr(gen):
                l, m = (2, gen) if gen < 16 else (3, gen - 16)
                w8t = pwdr.tile([128, KH * 128], F8E4, tag="wdr", name=f"wdr_{l}_{m}")
                nc.scalar.dma_start(w8t[:], wl_d[l][:, m * 2048:(m + 1) * 2048])
                wdr_pf[(l, m)] = w8t

            for gen in range(3):
                emit_wdr(gen)

            w4f = pconst.tile([128, C * KH], F16, tag="w4f")
            nc.scalar.dma_start(w4f[:], w4pk_d)

            parts = {}
            gchunk = {}     # (l, ci) -> allreduced pre-reduced stats tile
            stats = {}      # (l, ci) -> dict of stat tiles
            arouts = {}

            def emit_ar_fire(l, ci, do_sq):
                """n-pair-reduce sums, append sq cols -> DRAM -> AllReduce."""
                bd = _bounds(l)
                c0, c1 = bd[ci], bd[ci + 1]
                nm = c1 - c0
                w = 2 * nm if do_sq else nm
                red = pstat.tile([128, nm], F32, tag=f"red{l}{ci}", name=f"red{l}{ci}")
                nc.vector.tensor_reduce(
                    red[:],
                    parts[l][:, 2 * c0:2 * c1].rearrange("p (m n) -> p m n", n=2),
                    axis=mybir.AxisListType.X, op=ALU.add)
                arin = pdram.tile([128, w], F32, tag=f"arin{l}{ci}")
                arout = pdram.tile([128, w], F32, tag=f"arout{l}{ci}")
                nc.sync.dma_start(arin[:, 0:nm], red[:])
                if do_sq:
                    nc.sync.dma_start(arin[:, nm:w], parts[l][:, 32 + c0:32 + c1])
                nc.gpsimd.collective_compute(
                    "AllReduce", ALU.add, replica_groups=RG,
                    ins=[arin.opt()], outs=[arout.opt()])
                arouts[(l, ci)] = (arout, w)

            def emit_ar_land(l, ci):
                arout, w = arouts[(l, ci)]
                g_t = pstat.tile([128, w], F32, tag=f"g{l}{ci}", name=f"g{l}{ci}")
                nc.sync.dma_start(g_t[:], arout[:])
                gchunk[(l, ci)] = g_t

            def _st(l, ci, tag, nm):
                return pstat.tile([128, nm], F32, tag=f"{tag}{l}{ci}",
                                  name=f"{tag}{l}{ci}")

            def emit_stats_pre(l, ci, do_sq, fastl):
                """DVE-only stats from the pre-reduced AR result (safe to emit
                mid-loop: no ACT ops to block later drains)."""
                g_t = gchunk[(l, ci)]
                bd = _bounds(l)
                nm = bd[ci + 1] - bd[ci]
                m1 = _st(l, ci, "m1", nm)
                nc.vector.tensor_scalar_mul(m1[:], g_t[:, 0:nm], 1.0 / B)
                if fastl and not do_sq:
                    negm = _st(l, ci, "negm", nm)
                    nc.vector.tensor_scalar_mul(negm[:], g_t[:, 0:nm], -1.0 / B)
                    stats[(l, ci)] = dict(m1=m1, negm=negm, fast=True)
                    return
                msq, m1sq, v = (_st(l, ci, x, nm) for x in ("msq", "m1sq", "v"))
                nc.vector.tensor_scalar_mul(msq[:], g_t[:, nm:2 * nm], 1.0 / B)
                nc.vector.tensor_tensor(m1sq[:], m1[:], m1[:], op=ALU.mult)
                nc.vector.tensor_tensor(v[:], msq[:], m1sq[:], op=ALU.subtract)
                nc.vector.tensor_scalar_add(v[:], v[:], EPS)
                stats[(l, ci)] = dict(m1=m1, v=v, fast=False)

            def emit_stats_post(l, ci, fastl):
                """ACT sqrt + downstream scale/bias (emit after the layer's
                drains so the ACT queue never blocks on a pending AR)."""
                d = stats[(l, ci)]
                if d["fast"]:
                    return
                bd = _bounds(l)
                c0 = bd[ci]
                nm = bd[ci + 1] - c0
                gcol = cons[f"g{l}"][:, c0:c0 + nm]
                btcol = cons[f"bt{l}"][:, c0:c0 + nm]
                m1, v = d["m1"], d["v"]
                r, rp, mt, cc = (_st(l, ci, x, nm) for x in ("r", "rp", "mt", "c"))
                sq = _st(l, ci, "sq", nm)
                nc.scalar.activation(sq[:], v[:], ACT.Sqrt)
                nc.vector.reciprocal(r[:], sq[:])
                nc.vector.tensor_tensor(rp[:], gcol, r[:], op=ALU.mult)
                nc.vector.tensor_tensor(mt[:], m1[:], rp[:], op=ALU.mult)
                nc.vector.tensor_tensor(cc[:], btcol, mt[:], op=ALU.subtract)
                d.update(rp=rp, c=cc)
                if l < 3:
                    gi, u, u2, tthr, s, s2, sneg = (
                        _st(l, ci, x, nm)
                        for x in ("gi", "u", "u2", "tthr", "s", "s2", "sneg"))
                    nc.vector.reciprocal(gi[:], gcol)
                    nc.vector.tensor_tensor(u[:], btcol, gi[:], op=ALU.mult)
                    nc.vector.tensor_tensor(u2[:], u[:], sq[:], op=ALU.mult)
                    nc.vector.tensor_tensor(tthr[:], m1[:], u2[:], op=ALU.subtract)
                    nc.scalar.activation(s[:], gcol, ACT.Sign)
                    nc.vector.tensor_scalar_mul(s2[:], s[:], 2.0)
                    nc.vector.tensor_scalar_mul(sneg[:], s[:], -1.0)
                    d.update(tthr=tthr, s2=s2, sneg=sneg)

            def chunk_of(l, k):
                bd = _bounds(l)
                for ci in range(len(bd) - 1):
                    if k < bd[ci + 1]:
                        return ci, k - bd[ci]

            def sign_wave(l, dst3, h_t, krange, dve_only=False):
                """a[:, k, :] = sign-of-bn for k in krange; alternate ACT/DVE.
                dve_only for AR-end-gated chunks: keeps the ACT queue free of
                AR-gated ops so the next layer's drains + wdr DMA triggers
                (which ride the ACT queue in-order) are never blocked."""
                for k in krange:
                    ci, j = chunk_of(l, k)
                    s = stats[(l, ci)]
                    hsl = h_t[:, k * BS:(k + 1) * BS]
                    dst = dst3[:, k, :]
                    if k % 2 == 1 and not dve_only:
                        scale = 1.0 if s["fast"] else s["rp"][:, j:j + 1]
                        bias = s["negm"][:, j:j + 1] if s["fast"] else s["c"][:, j:j + 1]
                        nc.scalar.activation(dst, hsl, ACT.Sign, bias=bias, scale=scale)
                    elif s["fast"]:
                        # {0,1} encoding: weights are +-2 with bias correction
                        nc.vector.tensor_scalar(out=dst, in0=hsl,
                                                scalar1=s["m1"][:, j:j + 1],
                                                scalar2=None, op0=ALU.is_ge)
                    else:
                        thr = s["tthr"][:, j:j + 1]
                        bt_ = pscr.tile([128, BS], F16, tag="scr", name=f"sgb_{l}_{k}")
                        nc.vector.tensor_scalar(out=bt_[:], in0=hsl, scalar1=thr,
                                                scalar2=None, op0=ALU.is_ge)
                        nc.vector.tensor_scalar(out=dst, in0=bt_[:],
                                                scalar1=s["s2"][:, j:j + 1],
                                                scalar2=s["sneg"][:, j:j + 1],
                                                op0=ALU.mult, op1=ALU.add)

            def drain(l, m, n, ps, h_t, do_sq):
                hs = h_t[:, m * BS + n * 512: m * BS + n * 512 + 512]
                col = 2 * m + n
                nc.scalar.activation(hs, ps[:], ACT.Identity,
                                     bias=cons[f"b{l}"][:, m:m + 1], scale=1.0,
                                     accum_out=parts[l][:, col:col + 1])

            def msum(l, m, h_t, do_sq):
                """L3 sumsq: one fused DVE op (the ACT-Square+accum pair was
                2.8us/m-tile and made L3 ACT-bound)."""
                if not do_sq:
                    return
                hrow = h_t[:, m * BS:(m + 1) * BS]
                scr = pscr.tile([128, BS], F32, tag="scr", name=f"sq_{l}_{m}")
                nc.vector.tensor_tensor(scr[:], hrow, hrow, op=ALU.mult)
                nc.vector.tensor_reduce(
                    parts[l][:, 32 + m:32 + m + 1],
                    scr[:].rearrange("p (a b) -> p a b", a=1),
                    axis=mybir.AxisListType.X, op=ALU.add)

            def debug_out(src_ap, cast=False):
                if cast:
                    t = pscr.tile([128, BS], F32, tag="scr", name="dbgcast")
                    nc.vector.tensor_copy(t[:C, :], src_ap)
                    src_ap = t[:C, :]
                nc.sync.dma_start(out_d[:], src_ap)

            # ===================== Layer 1 =====================
            h1 = ph.tile([128, KH * BS], F32, tag="ph", name="h1")
            parts[1] = pstat.tile([128, 64], F32, tag="parts1", name="parts1")
            do_sq1 = not fast[0]
            bd1 = _bounds(1)

            def l1_mtile_alloc(m):
                st = {}
                st["wst"] = w1_pf.pop(m)
                st["w8lo"] = pw8.tile([128, KD * 128], F8E5, tag="w8",
                                      name=f"w8lo_{m}")
                st["w8lov"] = st["w8lo"][:].rearrange("p (k c) -> p k c", c=128)
                st["pss"] = [ppsum.tile([128, 512], F32, tag="ps",
                                        name=f"ps1_{m}_{n}") for n in range(NB)]
                return st

            def l1_chunk(st, c):
                wst, w8lo, w8lov, pss = (st["wst"], st["w8lo"], st["w8lov"],
                                         st["pss"])
                for k in range(c * KC, (c + 1) * KC):
                    lhsT = wst[:, k * 128:(k + 1) * 128]
                    for n in range(NB):
                        nc.tensor.matmul(
                            pss[n][:], lhsT,
                            xhi[:, k * BS + n * 512: k * BS + n * 512 + 512],
                            start=(k == 0), stop=False)
                nc.vector.tensor_scalar_mul(
                    w8lo[:, c * KC * 128:(c + 1) * KC * 128],
                    wst[:, c * KC * 128:(c + 1) * KC * 128], 2.0 ** -12)
                for t in range(c * KC // 2, (c + 1) * KC // 2):
                    lhsT = w8lov[:, 2 * t:2 * t + 2, :]
                    for n in range(NB):
                        nc.tensor.matmul(
                            pss[n][:], lhsT,
                            xlo8v[:, 2 * t:2 * t + 2, n * 512:n * 512 + 512],
                            start=False, stop=(t == KD // 2 - 1), perf_mode=DRM)

            def l1_tail(m, st):
                # W1 prefetch 3 ahead; Wdr gens 3..7 during m=8..12
                if m + 3 < KH and m + 3 not in w1_pf:
                    w1_pf[m + 3] = pw.tile([128, KD * 128], F8E4, tag="w",
                                           name=f"w1_{m + 3}")
                    nc.scalar.dma_start(
                        w1_pf[m + 3][:],
                        w1_d[:, (m + 3) * KD * 128:(m + 4) * KD * 128])
                if 8 <= m <= 12:
                    emit_wdr(m - 5)
                for n in range(NB):
                    drain(1, m, n, st["pss"][n], h1, do_sq1)
                msum(1, m, h1, do_sq1)
                for ci in range(len(bd1) - 1):
                    if m == bd1[ci + 1] - 1:
                        emit_ar_fire(1, ci, do_sq1)
                        if ci > 0:
                            emit_ar_land(1, ci - 1)

            # m=0,1 ride the incoming DMA wave: hi-passes chunk-paced on xhi,
            # lo-passes after (xlo streams behind xhi)
            st01 = {m: l1_mtile_alloc(m) for m in (0, 1)}
            for c in range(XCH):
                for m in (0, 1):
                    st = st01[m]
                    for k in range(c * KC, (c + 1) * KC):
                        lhsT = st["wst"][:, k * 128:(k + 1) * 128]
                        for n in range(NB):
                            nc.tensor.matmul(
                                st["pss"][n][:], lhsT,
                                xhi[:, k * BS + n * 512: k * BS + n * 512 + 512],
                                start=(k == 0), stop=False)
            for m in (0, 1):
                st = st01[m]
                nc.vector.tensor_scalar_mul(st["w8lo"][:], st["wst"][:], 2.0 ** -12)
            for c in range(XCH):
                for m in (0, 1):
                    st = st01[m]
                    for t in range(c * KC // 2, (c + 1) * KC // 2):
                        lhsT = st["w8lov"][:, 2 * t:2 * t + 2, :]
                        for n in range(NB):
                            nc.tensor.matmul(
                                st["pss"][n][:], lhsT,
                                xlo8v[:, 2 * t:2 * t + 2, n * 512:n * 512 + 512],
                                start=False, stop=(t == KD // 2 - 1),
                                perf_mode=DRM)
            for m in (0, 1):
                l1_tail(m, st01[m])
            for m in range(2, KH):
                st = l1_mtile_alloc(m)
                for c in range(XCH):
                    l1_chunk(st, c)
                l1_tail(m, st)
            emit_ar_land(1, len(bd1) - 2)

            if stage == 1:
                debug_out(h1[:C, :BS])

            a2 = pa.tile([128, KH, BS], F8E4, tag="pa", name="a2")
            nch1 = len(bd1) - 1
            for ci in range(nch1):
                emit_stats_pre(1, ci, do_sq1, fast[0])
                emit_stats_post(1, ci, fast[0])
                sign_wave(1, a2, h1, range(bd1[ci], bd1[ci + 1]),
                          dve_only=(ci == nch1 - 1))
            if stage == 2:
                debug_out(a2[:C, 0, :], cast=True)

            # ===================== Layers 2, 3 =====================
            def dense_dr(l, a_in):
                h_t = ph.tile([128, KH * BS], F32, tag="ph", name=f"h{l}")
                parts[l] = pstat.tile([128, 64], F32, tag=f"parts{l}", name=f"parts{l}")
                do_sq = (l == 3) or not fast[l - 1]
                bd = _bounds(l)
                # t-phases matching the PRODUCING layer's sign chunks
                pb_in = _bounds(l - 1)
                tph = [(pb_in[i] // 2, pb_in[i + 1] // 2) for i in range(len(pb_in) - 1)]
                for q in range(KH // 4):
                    ms = range(4 * q, 4 * q + 4)
                    pss = {m: [ppsum.tile([128, 512], F32, tag="ps",
                                          name=f"ps{l}_{m}_{n}") for n in range(NB)]
                           for m in ms}
                    w8 = {m: wdr_pf.pop((l, m)) for m in ms}
                    for ta, tb in tph:
                        for m in ms:
                            w8v = w8[m][:].rearrange("p (k c) -> p k c", c=128)
                            for t in range(ta, tb):
                                lhsT = w8v[:, 2 * t:2 * t + 2, :]
                                for n in range(NB):
                                    nc.tensor.matmul(
                                        pss[m][n][:], lhsT,
                                        a_in[:, 2 * t:2 * t + 2, n * 512:n * 512 + 512],
                                        start=(t == 0), stop=(t == KH // 2 - 1),
                                        perf_mode=DRM)
                    for m in ms:
                        gen = (l - 2) * 16 + m + 8
                        if gen < 32:
                            emit_wdr(gen)
                        for n in range(NB):
                            drain(l, m, n, pss[m][n], h_t, do_sq)
                        msum(l, m, h_t, do_sq)
                        for ci in range(len(bd) - 1):
                            if m == bd[ci + 1] - 1:
                                emit_ar_fire(l, ci, do_sq)
                                if ci > 0:
                                    emit_ar_land(l, ci - 1)
                emit_ar_land(l, len(bd) - 2)
                return h_t

            if stage >= 3:
                h2 = dense_dr(2, a2[:])
                a3 = pb.tile([128, KH, BS], F8E4, tag="pb", name="a3")
                bd2 = _bounds(2)
                for ci in range(len(bd2) - 1):
                    emit_stats_pre(2, ci, not fast[1], fast[1])
                    emit_stats_post(2, ci, fast[1])
                    sign_wave(2, a3, h2, range(bd2[ci], bd2[ci + 1]),
                              dve_only=(ci >= 1))
                if stage == 3:
                    debug_out(a3[:C, 0, :], cast=True)

            if stage >= 4:
                h3 = dense_dr(3, a3[:])
                # y3 = clip(bn3(h3), -1, 1) in fp16; L4 matmuls follow per k
                logits = plog.tile([16, BS], F32, tag="logits")
                ps4 = [ppsum.tile([128, 512], F32, tag="ps", name=f"ps4_{n}")
                       for n in range(NB)]
                y3dbg = None
                bd3 = _bounds(3)
                for ci in range(len(bd3) - 1):
                    emit_stats_pre(3, ci, True, False)
                for ci in range(len(bd3) - 1):
                    emit_stats_post(3, ci, False)
                    s = stats[(3, ci)]
                    for k in range(bd3[ci], bd3[ci + 1]):
                        j = k - bd3[ci]
                        scr = pscr.tile([128, BS], F32, tag="scr", name=f"y3s_{k}")
                        if k % 4 == 3:
                            # full-DVE path: offloads the serial ACT chain
                            nc.vector.tensor_scalar(
                                out=scr[:], in0=h3[:, k * BS:(k + 1) * BS],
                                scalar1=s["rp"][:, j:j + 1],
                                scalar2=s["c"][:, j:j + 1],
                                op0=ALU.mult, op1=ALU.add)
                        else:
                            nc.scalar.activation(scr[:],
                                                 h3[:, k * BS:(k + 1) * BS],
                                                 ACT.Identity,
                                                 bias=s["c"][:, j:j + 1],
                                                 scale=s["rp"][:, j:j + 1])
                        y3k = py3.tile([128, BS], F16, tag="y3", name=f"y3_{k}")
                        nc.vector.tensor_scalar(out=y3k[:], in0=scr[:],
                                                scalar1=-1.0, scalar2=1.0,
                                                op0=ALU.max, op1=ALU.min)
                        if k == 0:
                            y3dbg = y3k
                        if stage >= 5:
                            for n in range(NB):
                                nc.tensor.matmul(
                                    ps4[n][:C, :], w4f[:, k * C:(k + 1) * C],
                                    y3k[:, n * 512:(n + 1) * 512],
                                    start=(k == 0), stop=(k == KH - 1))
                if stage == 4:
                    debug_out(y3dbg[:C, :], cast=True)

            if stage >= 5:
                # ===== logits + log-softmax, 4 chunks of 256 cols =====
                for qq in range(2):
                    bank = ps4[qq]
                    qsl = slice(qq * 512, (qq + 1) * 512)
                    # logits on DVE (PSUM read) in parallel with exp on ACT
                    nc.vector.tensor_scalar(out=logits[:C, qsl], in0=bank[:C, :],
                                            scalar1=b4s[:C, :], scalar2=None,
                                            op0=ALU.add)
                    e_q = ptail.tile([16, 512], F32, tag="tl", name=f"e_{qq}")
                    nc.scalar.activation(e_q[:C, :], bank[:C, :], ACT.Exp,
                                         bias=b4s[:C, :], scale=1.0)
                    ps5 = ppsum.tile([128, 512], F32, tag="ps", name=f"ps5_{qq}")
                    nc.tensor.matmul(ps5[:1, :], ones10[:C, :], e_q[:C, :],
                                     start=True, stop=True)
                    lse_q = ptail.tile([16, 512], F32, tag="tl", name=f"lse_{qq}")
                    nc.scalar.activation(lse_q[:1, :], ps5[:1, :], ACT.Ln)
                    ps6 = ppsum.tile([128, 512], F32, tag="ps", name=f"ps6_{qq}")
                    nc.tensor.matmul(ps6[:C, :], onesC[:1, :C], lse_q[:1, :],
                                     start=True, stop=True)
                    outs_q = ptail.tile([16, 512], F32, tag="tl", name=f"o_{qq}")
                    nc.vector.tensor_tensor(outs_q[:C, :], logits[:C, qsl],
                                            ps6[:C, :], op=ALU.subtract)
                    if stage >= 6:
                        nc.sync.dma_start(out_d[:, qsl], outs_q[:C, :])
                if stage == 5:
                    debug_out(logits[:C, :])

    nc.compile()
    return nc


def _prep_inputs(x, W1, b1, g1, bt1, W2, b2, g2, bt2, W3, b3, g3, bt3, W4, b4):
    """Host-side sharding + layout prep (sign, fp8 cast, p-major packing)."""
    def as32(a):
        return np.ascontiguousarray(np.asarray(a, dtype=np.float32))

    f8 = ml_dtypes.float8_e4m3

    def sgn(w):
        return np.where(np.asarray(w) >= 0, np.float32(1.0), np.float32(-1.0))

    def pack_w(w, kt, uks=()):
        # [H_out, K] -> [128, (H_out/128) * K] with per-m-tile p-major blocks.
        # uks: k-blocks whose activations come {0,1}-encoded -> weights +-2.
        s = sgn(w).reshape(-1, 128, kt, 128)            # [m, c, k, p]
        if uks:
            s[:, :, sorted(uks), :] *= 2.0
        s = s.transpose(0, 3, 2, 1).reshape(s.shape[0], 128, kt * 128)
        return np.ascontiguousarray(
            s.transpose(1, 0, 2).reshape(128, -1)).astype(f8)

    def ok(g, bt):
        g, bt = np.asarray(g), np.asarray(bt)
        return bool(not np.any(bt) and np.all(g > 0))

    def ucorr(w, uks):
        # bias correction: -sum over u-encoded k-blocks of sign(w)
        if not uks:
            return 0.0
        s = sgn(w).reshape(w.shape[0], -1, 128)
        return s[:, sorted(uks), :].sum(axis=(1, 2))

    x = as32(x)
    u2 = _u_ks(1) if ok(g1, bt1) else set()
    u3 = _u_ks(2) if ok(g2, bt2) else set()
    W2, W3, b2, b3 = as32(W2), as32(W3), as32(b2), as32(b3)
    shared = {
        "w1pk": pack_w(as32(W1), KD),
        "w2pk": pack_w(W2, KH, u2),
        "w3pk": pack_w(W3, KH, u3),
    }
    b2 = b2 - ucorr(W2, u2)
    b3 = b3 - ucorr(W3, u3)
    cvecs = (b1, g1, bt1, b2, g2, bt2, b3, g3, bt3)
    cpk = np.empty((128, KH * len(cvecs)), np.float32)
    for i, v in enumerate(cvecs):
        cpk[:, i * KH:(i + 1) * KH] = as32(v).reshape(KH, 128).T
    shared["cpk"] = cpk
    w4T = np.ascontiguousarray(as32(W4).T)          # [H, C]
    w4pk = np.empty((128, C * KH), np.float16)
    for k in range(KH):
        w4pk[:, k * C:(k + 1) * C] = w4T[k * 128:(k + 1) * 128, :].astype(np.float16)
    shared["w4pk"] = w4pk
    b4p = np.zeros((16, 1), np.float32)
    b4p[:C, 0] = as32(b4).reshape(-1)
    shared["c_b4"] = b4p

    in_maps = []
    for cr in range(NCORES):
        xT = np.ascontiguousarray(x[cr * BS:(cr + 1) * BS].T)     # [D, BS]
        hi = xT.astype(np.float16)
        lo8 = ((xT - hi.astype(np.float32)) * 4096.0).astype(f8)
        # p-major pack: [D, BS] -> [128, KD*BS]
        hi_pk = np.ascontiguousarray(
            hi.reshape(KD, 128, BS).transpose(1, 0, 2).reshape(128, KD * BS))
        lo_pk = np.ascontiguousarray(
            lo8.reshape(KD, 128, BS).transpose(1, 0, 2).reshape(128, KD * BS))
        m = dict(shared)
        m["xT_hi"] = hi_pk
        m["xT_lo8"] = lo_pk
        in_maps.append(m)
    return in_maps


def _fast_flags(inputs):
    """Mean-only BN boundary valid when beta==0 and gamma>0."""
    def ok(g, bt):
        g, bt = np.asarray(g), np.asarray(bt)
        return bool(not np.any(bt) and np.all(g > 0))

    return (ok(inputs["g1"], inputs["bt1"]), ok(inputs["g2"], inputs["bt2"]))


def kernel(**inputs) -> np.ndarray:
    from concourse.bass_utils import run_bass_kernel_spmd

    fast = _fast_flags(inputs)
    if _CACHE.get("fast") != fast:
        _CACHE["nc"] = _build(fast=fast)
        _CACHE["fast"] = fast
    nc = _CACHE["nc"]
    in_maps = _prep_inputs(**inputs)
    res = run_bass_kernel_spmd(nc, in_maps, list(range(NCORES)))
    out = np.concatenate([res.results[c]["outT"].T for c in range(NCORES)], axis=0)
    return out.astype(np.float32)
